# revision 13
# baseline (speedup 1.0000x reference)
"""GAT-style 2-conv GNN forward on 8 Trainium2 NeuronCores.

Strategy (graph/data parallel): nodes partitioned across 8 cores by
destination range; per-edge source-row gathers via InstDMAGatherAnt (int16
indices) against a replicated node-feature table.

Perf notes vs. the first working version (3.14 ms):
  - dma_gather costs ~7.8 ns per index of serialized GPSIMD time; total
    gathered indices are the kernel's critical resource.
  - d-values (per-dst attention bias) are RECOMPUTED per window order from
    host-permuted x on the idle TensorEngine instead of being gathered
    (-50k indices/core).
  - Windows are local row-quarters (not core pairs), so the table AllGather
    splits into 4 sub-collectives pipelined against window processing.
  - Pad rows are distributed per-quarter so each window table has dummy
    rows for grid padding.
"""

import sys

sys.path.insert(0, "/opt/trn_rl_repo")

import contextlib

import numpy as np

import concourse.bacc as bacc
import concourse.bass as bass
import concourse.bass_utils as bass_utils
import concourse.mybir as mybir
import concourse.tile as tile
from concourse import library_config
from concourse.masks import make_identity

FP32 = mybir.dt.float32
FP16 = mybir.dt.float16
INT16 = mybir.dt.int16

N_CORES = 8
N_WIN = 4
P = 128
ELEM = 128  # fp16 cols per table row = 256B

# table row columns
S1C, S2C, D1C, D2C = 32, 65, 66, 67
H1B, H2B = 0, 33
# partial row columns
PN1, PN2, PD1, PD2 = 0, 32, 64, 65
NEG_SLOPE = 0.2
DUMMY_S = -30000.0

S_CHUNK = 96  # grid columns per chunk (12288 idxs: descriptor-ring limit)
QTILES = (25, 25, 24, 24)  # tiles per window-quarter (sum = 98)


def _wrap16(stream):
    """[n] -> [128, n//16] int16 in the 16-partition wrapped+replicated layout."""
    n = stream.shape[0]
    assert n % 16 == 0
    w = stream.reshape(n // 16, 16).T.astype(np.int16)  # [16, n//16]
    return np.tile(w, (8, 1))


def _build_layout(edge_index, n_nodes):
    src = np.asarray(edge_index[0], dtype=np.int64)
    dst = np.asarray(edge_index[1], dtype=np.int64)
    E = src.shape[0]

    npc_raw = -(-n_nodes // N_CORES)
    tiles = -(-npc_raw // P)
    npc = tiles * P
    if npc == npc_raw:  # ensure pad rows exist
        tiles += 1
        npc += P
    n_pads = npc - npc_raw
    assert tiles == sum(QTILES)

    # quarter row layout: each quarter gets some pad rows at its end
    qrows = np.array(QTILES, dtype=np.int64) * P          # rows per quarter
    qrow_start = np.concatenate([[0], np.cumsum(qrows)])  # [5]
    padq = np.full(N_WIN, n_pads // N_WIN, dtype=np.int64)
    padq[: n_pads % N_WIN] += 1
    qreal = qrows - padq                                   # real rows/quarter
    qreal_start = np.concatenate([[0], np.cumsum(qreal)])  # over raw index
    assert qreal_start[-1] == npc_raw

    wsize = (qrows * N_CORES).astype(np.int64)             # table rows/window
    assert (wsize <= 32768).all()
    pad_rel = qreal.copy()  # core-0 pad start, window-relative

    # old local offset -> new local row
    off = np.arange(npc_raw)
    oq = np.searchsorted(qreal_start[1:], off, side="right")
    new_local = qrow_start[oq] + (off - qreal_start[oq])

    old2new = np.empty(n_nodes, dtype=np.int64)
    for c in range(N_CORES):
        lo = c * npc_raw
        hi = min(lo + npc_raw, n_nodes)
        old2new[lo:hi] = c * npc + new_local[: hi - lo]

    new_src = old2new[src]
    new_dst = old2new[dst]
    dst_core = new_dst // npc
    dst_local = new_dst % npc
    src_core = new_src // npc
    src_local = new_src % npc
    src_win = np.searchsorted(qrow_start[1:4], src_local, side="right")
    # window-table index of each edge's source
    src_tab = src_core * qrows[src_win] + (src_local - qrow_start[src_win])

    # per (core, window) in-degree
    qdeg = np.zeros((N_CORES, N_WIN, npc), dtype=np.int64)
    np.add.at(qdeg, (dst_core, src_win, dst_local), 1)

    node_at = np.empty((N_CORES, N_WIN, npc), dtype=np.int64)
    pos_of = np.empty((N_CORES, N_WIN, npc), dtype=np.int64)
    for c in range(N_CORES):
        for q in range(N_WIN):
            o = np.argsort(-qdeg[c, q], kind="stable")
            node_at[c, q] = o
            pos_of[c, q, o] = np.arange(npc)

    # shared tile degree profile per window
    D_q = np.zeros((N_WIN, tiles), dtype=np.int64)
    for q in range(N_WIN):
        sorted_deg = np.take_along_axis(qdeg[:, q, :], node_at[:, q, :], axis=1)
        D_q[q] = sorted_deg[:, ::P].max(axis=0)

    fb_q = np.zeros((N_WIN, tiles + 1), dtype=np.int64)
    for q in range(N_WIN):
        fb_q[q, 1:] = np.cumsum(D_q[q])
    slots_q = fb_q[:, -1].copy()

    # edge -> grid cell
    pos = pos_of[dst_core, src_win, dst_local]
    t = pos // P
    p = pos % P
    key = (dst_core * N_WIN + src_win) * npc + dst_local
    order = np.argsort(key, kind="stable")
    sk = key[order]
    first = np.flatnonzero(np.r_[True, sk[1:] != sk[:-1]])
    group_start = np.repeat(first, np.diff(np.r_[first, E]))
    j = np.empty(E, dtype=np.int64)
    j[order] = np.arange(E) - group_start
    assert (j < D_q[src_win, t]).all()
    col = fb_q[src_win, t] + j

    gidx = np.empty((N_CORES, P, int(slots_q.sum()) * 8), dtype=np.int16)
    wbase = np.concatenate([[0], np.cumsum(slots_q)])
    for c in range(N_CORES):
        for q in range(N_WIN):
            sq = int(slots_q[q])
            stream = np.full(sq * P, pad_rel[q], dtype=np.int64)
            m = (dst_core == c) & (src_win == q)
            stream[col[m] * P + p[m]] = src_tab[m]
            assert stream.max() < wsize[q] and stream.min() >= 0
            gidx[c, :, int(wbase[q]) * 8 : int(wbase[q] + sq) * 8] = _wrap16(
                stream
            )

    # merge-gather index streams (partial q-order -> common order)
    mgidx = np.empty((N_CORES, P, N_WIN * npc // 16), dtype=np.int16)
    for c in range(N_CORES):
        for q in range(N_WIN):
            sl = slice(q * npc // 16, (q + 1) * npc // 16)
            mgidx[c, :, sl] = _wrap16(pos_of[c, q])

    # chunk structure per window
    win_chunks = []
    for q in range(N_WIN):
        runs = []
        t0 = 0
        for tt in range(1, tiles + 1):
            if tt == tiles or D_q[q, tt] != D_q[q, t0]:
                if D_q[q, t0] > 0:
                    runs.append((t0, tt - t0, int(D_q[q, t0])))
                t0 = tt
        pieces = []
        for (rt0, g, d) in runs:
            max_g = max(1, S_CHUNK // d)
            s = 0
            while s < g:
                gg = min(max_g, g - s)
                pieces.append((rt0 + s, gg, d))
                s += gg
        chunks = []
        cur, cur_cols = [], 0
        for pc in pieces:
            need = pc[1] * pc[2]
            assert need <= S_CHUNK
            if cur_cols + need > S_CHUNK:
                chunks.append(cur)
                cur, cur_cols = [], 0
            cur.append(pc)
            cur_cols += need
        if cur:
            chunks.append(cur)
        win_chunks.append(chunks)

    # pad-row mask in common order [P, tiles]
    padm = np.zeros((npc,), dtype=np.float16)
    for q in range(N_WIN):
        padm[qrow_start[q] + qreal[q] : qrow_start[q + 1]] = DUMMY_S
    padm = np.ascontiguousarray(padm.reshape(tiles, P).T)

    return dict(
        npc_raw=npc_raw, npc=npc, tiles=tiles, wsize=wsize, qrows=qrows,
        qrow_start=qrow_start, old2new=old2new, D_q=D_q, fb_q=fb_q,
        slots_q=slots_q, gidx=gidx, mgidx=mgidx, win_chunks=win_chunks,
        node_at=node_at, padm=padm,
    )


def _build_program(lay, f_in, hidden, ncls):
    tiles = lay["tiles"]
    npc = lay["npc"]
    wsize = lay["wsize"]
    qrow_start = lay["qrow_start"]
    slots_q = lay["slots_q"]
    fb_q = lay["fb_q"]
    win_chunks = lay["win_chunks"]
    F = 2 * ncls + 1
    assert F == f_in
    HC = 2 * ncls + 4

    nc = bacc.Bacc("TRN2", target_bir_lowering=False, debug=False,
                   enable_asserts=False, num_devices=N_CORES,
                   num_swdge_queues=2)

    xT_in = nc.dram_tensor("xT", [f_in, npc], FP32, kind="ExternalInput").ap()
    xTq_in = [
        nc.dram_tensor(f"xTq{q}", [f_in, npc], FP32, kind="ExternalInput").ap()
        for q in range(N_WIN)
    ]
    x_in = nc.dram_tensor("xrow", [npc, f_in], FP32, kind="ExternalInput").ap()
    wmlp_in = nc.dram_tensor("wmlp", [f_in, hidden], FP32, kind="ExternalInput").ap()
    bmlp_in = nc.dram_tensor("bmlp", [hidden, 1], FP32, kind="ExternalInput").ap()
    wcat_in = nc.dram_tensor("wcat", [hidden, HC], FP32, kind="ExternalInput").ap()
    wd_in = nc.dram_tensor("wd", [hidden, 2], FP32, kind="ExternalInput").ap()
    bb_in = nc.dram_tensor("bb", [P, 2 * ncls], FP32, kind="ExternalInput").ap()
    padm_in = nc.dram_tensor("padm", [P, tiles], FP16, kind="ExternalInput").ap()
    gidx_in = nc.dram_tensor(
        "gidx", [P, int(slots_q.sum()) * 8], INT16, kind="ExternalInput"
    ).ap()
    mgidx_in = nc.dram_tensor(
        "mgidx", [P, N_WIN * npc // 16], INT16, kind="ExternalInput"
    ).ap()
    out_t = nc.dram_tensor("out", [npc, F], FP32, kind="ExternalOutput").ap()

    with tile.TileContext(nc) as tc:
        with contextlib.ExitStack() as ctx:
            persist = ctx.enter_context(tc.tile_pool(name="persist", bufs=1))
            dram = ctx.enter_context(tc.tile_pool(name="dram", bufs=1, space="DRAM"))
            cpool = ctx.enter_context(tc.tile_pool(name="consts", bufs=1))

            nc.gpsimd.load_library(library_config.mlp)

            x3buf = persist.tile([P, tiles], FP32)
            bb_sb = persist.tile([P, 2 * ncls], FP32)
            dq_sb = persist.tile([P, N_WIN, tiles, 2], FP32)
            nc.sync.dma_start(out=bb_sb[:], in_=bb_in[:])

            hloc_d = [
                dram.tile([int(lay["qrows"][q]), ELEM], FP16,
                          name=f"hloc{q}", tag=f"hloc{q}")
                for q in range(N_WIN)
            ]
            htab_d = [
                dram.tile([int(wsize[q]), ELEM], FP16, name=f"htab{q}",
                          tag=f"htab{q}")
                for q in range(N_WIN)
            ]
            part_d = [dram.tile([npc, ELEM], FP16, name=f"part{q}",
                                tag=f"part{q}")
                      for q in range(N_WIN)]

            wmlp_sb = cpool.tile([f_in, hidden], FP32)
            bmlp_sb = cpool.tile([hidden, 1], FP32)
            wd_sb = cpool.tile([hidden, 2], FP32)
            ident = cpool.tile([P, P], FP32)
            nc.sync.dma_start(out=wmlp_sb[:], in_=wmlp_in[:])
            nc.sync.dma_start(out=bmlp_sb[:], in_=bmlp_in[:])
            nc.sync.dma_start(out=wd_sb[:], in_=wd_in[:])
            make_identity(nc, ident[:])

            # ---------------- Phase 1: dense local features ----------------
            with tc.tile_pool(name="ph1c", bufs=1) as c1pool, \
                 tc.tile_pool(name="ph1", bufs=3) as ph1, \
                 tc.tile_pool(name="ph1x", bufs=2) as ph1x, \
                 tc.tile_pool(name="hl", bufs=1) as hlp, \
                 tc.tile_pool(name="ps1", bufs=2, space="PSUM") as ps1, \
                 tc.tile_pool(name="ps2", bufs=2, space="PSUM") as ps2, \
                 tc.tile_pool(name="ps3", bufs=2, space="PSUM") as ps3:
                wcat_sb = c1pool.tile([hidden, HC], FP32)
                padm_sb = c1pool.tile([P, tiles], FP16)
                nc.sync.dma_start(out=wcat_sb[:], in_=wcat_in[:])
                nc.sync.dma_start(out=padm_sb[:], in_=padm_in[:])

                hq_sb = [
                    hlp.tile([P, QTILES[q], ELEM], FP16, name=f"hq{q}",
                             tag=f"hq{q}")
                    for q in range(N_WIN)
                ]
                for q in range(N_WIN):
                    nc.vector.memset(hq_sb[q][:], 0.0)

                XCH = 16
                q_next = 0
                for t in range(tiles):
                    if t % XCH == 0:
                        g = min(XCH, tiles - t)
                        xt_sb = ph1x.tile([f_in, XCH * P], FP32, tag="xt")
                        nc.sync.dma_start(
                            out=xt_sb[:, : g * P],
                            in_=xT_in[:, t * P : (t + g) * P],
                        )
                    xoff = (t % XCH) * P
                    psA = ps1.tile([P, P], FP32, space="PSUM")
                    nc.tensor.matmul(
                        out=psA[:], lhsT=wmlp_sb[:],
                        rhs=xt_sb[:, xoff : xoff + P],
                        start=True, stop=True,
                    )
                    x0t = ph1.tile([P, P], FP32, tag="x0t")
                    nc.scalar.activation(
                        out=x0t[:], in_=psA[:],
                        func=mybir.ActivationFunctionType.Relu,
                        bias=bmlp_sb[:, 0:1], scale=1.0,
                    )
                    psH = ps2.tile([P, HC], FP32, space="PSUM")
                    nc.tensor.matmul(
                        out=psH[:], lhsT=x0t[:], rhs=wcat_sb[:],
                        start=True, stop=True,
                    )
                    nc.vector.tensor_copy(
                        out=hq_sb[q_next][:, t - int(qrow_start[q_next] // P), 0:HC],
                        in_=psH[:],
                    )
                    psT = ps3.tile([P, P], FP32, space="PSUM")
                    nc.tensor.transpose(out=psT[:], in_=x0t[:], identity=ident[:])
                    nc.vector.tensor_reduce(
                        out=x3buf[:, t : t + 1], in_=psT[:],
                        axis=mybir.AxisListType.X, op=mybir.AluOpType.max,
                    )
                    # quarter complete -> mask pads, write to HBM
                    if t + 1 == qrow_start[q_next + 1] // P:
                        tb = int(qrow_start[q_next] // P)
                        te = t + 1
                        for scol in (S1C, S2C):
                            nc.vector.tensor_tensor(
                                out=hq_sb[q_next][:, :, scol : scol + 1],
                                in0=hq_sb[q_next][:, :, scol : scol + 1],
                                in1=padm_sb[:, tb:te].unsqueeze(2),
                                op=mybir.AluOpType.add,
                            )
                        nc.sync.dma_start(
                            out=hloc_d[q_next][:].rearrange(
                                "(t p) c -> p t c", p=P
                            ),
                            in_=hq_sb[q_next][:],
                        )
                        q_next += 1

            # ------------- Phase 2: recompute d per window order -----------
            with tc.tile_pool(name="ph2x", bufs=2) as ph2x, \
                 tc.tile_pool(name="ph2", bufs=3) as ph2, \
                 tc.tile_pool(name="psA2", bufs=2, space="PSUM") as psa2, \
                 tc.tile_pool(name="psD2", bufs=2, space="PSUM") as psd2:
                XCH = 16
                for q in range(N_WIN):
                    for t in range(tiles):
                        if t % XCH == 0:
                            g = min(XCH, tiles - t)
                            xq_sb = ph2x.tile([f_in, XCH * P], FP32, tag="xq")
                            nc.sync.dma_start(
                                out=xq_sb[:, : g * P],
                                in_=xTq_in[q][:, t * P : (t + g) * P],
                            )
                        xoff = (t % XCH) * P
                        psA = psa2.tile([P, P], FP32, space="PSUM")
                        nc.tensor.matmul(
                            out=psA[:], lhsT=wmlp_sb[:],
                            rhs=xq_sb[:, xoff : xoff + P],
                            start=True, stop=True,
                        )
                        x0q = ph2.tile([P, P], FP32, tag="x0q")
                        nc.scalar.activation(
                            out=x0q[:], in_=psA[:],
                            func=mybir.ActivationFunctionType.Relu,
                            bias=bmlp_sb[:, 0:1], scale=1.0,
                        )
                        psD = psd2.tile([P, 2], FP32, space="PSUM")
                        nc.tensor.matmul(
                            out=psD[:], lhsT=x0q[:], rhs=wd_sb[:],
                            start=True, stop=True,
                        )
                        nc.vector.tensor_copy(
                            out=dq_sb[:, q, t, :], in_=psD[:]
                        )

            # ---------------- Phase 3: per-window gather + reduce ----------
            with tc.tile_pool(name="gi", bufs=1) as gip, \
                 tc.tile_pool(name="msg", bufs=2) as msgp, \
                 tc.tile_pool(name="sc", bufs=2) as scp, \
                 tc.tile_pool(name="exb", bufs=2) as exp_, \
                 tc.tile_pool(name="pt", bufs=1) as ptp:
                gidx_sb = gip.tile([P, int(slots_q.sum()) * 8], INT16)
                nc.sync.dma_start(out=gidx_sb[:], in_=gidx_in[:])
                wbase = np.concatenate([[0], np.cumsum(slots_q)]).astype(int)

                def _trigger_ag(q):
                    nc.gpsimd.collective_compute(
                        "AllGather",
                        mybir.AluOpType.bypass,
                        replica_groups=[list(range(N_CORES))],
                        ins=[hloc_d[q][:].opt()],
                        outs=[htab_d[q][:].opt()],
                    )

                _trigger_ag(0)
                for q in range(N_WIN):
                    d1q = dq_sb[:, q, :, 0]
                    d2q = dq_sb[:, q, :, 1]

                    partial = ptp.tile([P, tiles, ELEM], FP16, tag="partial")
                    nc.vector.memset(partial[:], 0.0)

                    tab = htab_d[q][:]
                    for ci, chunk in enumerate(win_chunks[q]):
                        ccols = sum(g * d for (_, g, d) in chunk)
                        cb = int(fb_q[q, chunk[0][0]])
                        msg = msgp.tile([P, S_CHUNK, ELEM], FP16, tag="msg")
                        ib = (int(wbase[q]) + cb) * 8
                        nc.gpsimd.dma_gather(
                            out_ap=msg[:, :ccols, :],
                            in_ap=tab,
                            idxs_ap=gidx_sb[:, ib : ib + ccols * 8],
                            num_idxs=ccols * P,
                            num_idxs_reg=ccols * P,
                            elem_size=ELEM,
                            single_packet=False,
                            queue_num=ci % 2,
                        )
                        if ci == 0 and q + 1 < N_WIN:
                            _trigger_ag(q + 1)
                        for (rt0, g, d) in chunk:
                            s0 = int(fb_q[q, rt0]) - cb
                            mv = msg[:, s0 : s0 + g * d, :].rearrange(
                                "p (g e) c -> p g e c", e=d
                            )
                            for (hb, sc_, dq_, nb, db) in (
                                (H1B, S1C, d1q, PN1, PD1),
                                (H2B, S2C, d2q, PN2, PD2),
                            ):
                                exb = exp_.tile([P, S_CHUNK], FP32, tag="exb")
                                exv = exb[:, : g * d].rearrange(
                                    "p (g e) -> p g e", e=d
                                )
                                nc.vector.tensor_tensor(
                                    out=exv,
                                    in0=mv[:, :, :, sc_],
                                    in1=dq_[:, rt0 : rt0 + g]
                                    .unsqueeze(2)
                                    .broadcast_to([P, g, d]),
                                    op=mybir.AluOpType.add,
                                )
                                lrb = exp_.tile([P, S_CHUNK], FP32, tag="lrb")
                                lrv = lrb[:, : g * d].rearrange(
                                    "p (g e) -> p g e", e=d
                                )
                                nc.vector.tensor_scalar_mul(lrv, exv, NEG_SLOPE)
                                nc.vector.tensor_tensor(
                                    out=exv, in0=exv, in1=lrv,
                                    op=mybir.AluOpType.max,
                                )
                                nc.scalar.activation(
                                    out=exv, in_=exv,
                                    func=mybir.ActivationFunctionType.Exp,
                                )
                                sct = scp.tile(
                                    [P, S_CHUNK, ncls], FP32, tag="sc"
                                )
                                scv = sct[:, : g * d, :].rearrange(
                                    "p (g e) c -> p g e c", e=d
                                )
                                nc.vector.tensor_tensor(
                                    out=scv,
                                    in0=mv[:, :, :, hb : hb + ncls],
                                    in1=exv.unsqueeze(3).broadcast_to(
                                        [P, g, d, ncls]
                                    ),
                                    op=mybir.AluOpType.mult,
                                )
                                with nc.allow_low_precision("fp16 partials"):
                                    nc.vector.tensor_reduce(
                                        out=partial[:, rt0 : rt0 + g, nb : nb + ncls],
                                        in_=sct[:, : g * d, :].rearrange(
                                            "p (g e) c -> p g c e", e=d
                                        ),
                                        axis=mybir.AxisListType.X,
                                        op=mybir.AluOpType.add,
                                    )
                                    nc.vector.tensor_reduce(
                                        out=partial[:, rt0 : rt0 + g, db],
                                        in_=exv,
                                        axis=mybir.AxisListType.X,
                                        op=mybir.AluOpType.add,
                                    )
                    nc.sync.dma_start(
                        out=part_d[q][:].rearrange("(t p) c -> p t c", p=P),
                        in_=partial[:],
                    )

            # ---------------- Phase 3.9: merge partials --------------------
            with tc.tile_pool(name="mg", bufs=2) as mgp, \
                 tc.tile_pool(name="mgi", bufs=1) as mgip, \
                 tc.tile_pool(name="acc", bufs=1) as accp, \
                 tc.tile_pool(name="fin", bufs=1) as finp, \
                 tc.tile_pool(name="tmp", bufs=1) as tmpp:
                mgidx_sb = mgip.tile([P, N_WIN * npc // 16], INT16)
                nc.sync.dma_start(out=mgidx_sb[:], in_=mgidx_in[:])
                acc = accp.tile([P, tiles, 2 * ncls + 2], FP32)
                for q in range(N_WIN):
                    mg = mgp.tile([P, tiles, ELEM], FP16, tag="mg")
                    ib = q * npc // 16
                    nc.gpsimd.dma_gather(
                        out_ap=mg[:],
                        in_ap=part_d[q][:],
                        idxs_ap=mgidx_sb[:, ib : ib + npc // 16],
                        num_idxs=npc,
                        num_idxs_reg=npc,
                        elem_size=ELEM,
                        single_packet=False,
                        queue_num=q % 2,
                    )
                    if q == 0:
                        nc.vector.tensor_copy(
                            out=acc[:], in_=mg[:, :, 0 : 2 * ncls + 2]
                        )
                    else:
                        nc.vector.tensor_tensor(
                            out=acc[:], in0=acc[:],
                            in1=mg[:, :, 0 : 2 * ncls + 2],
                            op=mybir.AluOpType.add,
                        )

                # ------------- Phase 4: normalize + residual + lsm ---------
                xin = finp.tile([P, tiles, F], FP32)
                nc.sync.dma_start(
                    out=xin[:], in_=x_in[:].rearrange("(t p) f -> p t f", p=P)
                )
                rden = tmpp.tile([P, tiles], FP32, tag="rden")
                for conv in range(2):
                    numv = acc[:, :, conv * ncls : (conv + 1) * ncls]
                    denv = acc[:, :, 2 * ncls + conv]
                    nc.vector.tensor_scalar_add(denv, denv, 1e-16)
                    nc.vector.reciprocal(out=rden[:], in_=denv)
                    nc.vector.tensor_tensor(
                        out=numv, in0=numv,
                        in1=rden[:].unsqueeze(2).broadcast_to([P, tiles, ncls]),
                        op=mybir.AluOpType.mult,
                    )
                    nc.vector.tensor_tensor(
                        out=numv, in0=numv,
                        in1=bb_sb[:, conv * ncls : (conv + 1) * ncls]
                        .unsqueeze(1)
                        .broadcast_to([P, tiles, ncls]),
                        op=mybir.AluOpType.add,
                    )
                    if conv == 0:
                        nc.vector.tensor_scalar_max(numv, numv, 0.0)
                    nc.vector.tensor_tensor(
                        out=xin[:, :, conv * ncls : (conv + 1) * ncls],
                        in0=xin[:, :, conv * ncls : (conv + 1) * ncls],
                        in1=numv,
                        op=mybir.AluOpType.add,
                    )
                nc.vector.tensor_tensor(
                    out=xin[:, :, 2 * ncls], in0=xin[:, :, 2 * ncls],
                    in1=x3buf[:], op=mybir.AluOpType.add,
                )
                mx = tmpp.tile([P, tiles], FP32, tag="mx")
                nc.vector.tensor_reduce(
                    out=mx[:], in_=xin[:], axis=mybir.AxisListType.X,
                    op=mybir.AluOpType.max,
                )
                nc.vector.tensor_tensor(
                    out=xin[:], in0=xin[:],
                    in1=mx[:].unsqueeze(2).broadcast_to([P, tiles, F]),
                    op=mybir.AluOpType.subtract,
                )
                et = tmpp.tile([P, tiles, F], FP32, tag="et")
                nc.scalar.activation(
                    out=et[:], in_=xin[:],
                    func=mybir.ActivationFunctionType.Exp,
                )
                sm = tmpp.tile([P, tiles], FP32, tag="sm")
                nc.vector.tensor_reduce(
                    out=sm[:], in_=et[:], axis=mybir.AxisListType.X,
                    op=mybir.AluOpType.add,
                )
                lg = tmpp.tile([P, tiles], FP32, tag="lg")
                nc.scalar.activation(
                    out=lg[:], in_=sm[:],
                    func=mybir.ActivationFunctionType.Ln,
                )
                nc.vector.tensor_tensor(
                    out=xin[:], in0=xin[:],
                    in1=lg[:].unsqueeze(2).broadcast_to([P, tiles, F]),
                    op=mybir.AluOpType.subtract,
                )
                nc.sync.dma_start(
                    out=out_t[:].rearrange("(t p) f -> p t f", p=P), in_=xin[:]
                )

    nc.compile()
    return nc


def _run(nc, lay, x, W_mlp, b_mlp, W1, a1_src, a1_dst, b1,
         W2, a2_src, a2_dst, b2, trace=False):
    n_nodes, f_in = x.shape
    hidden = W_mlp.shape[1]
    ncls = W1.shape[1]
    npc = lay["npc"]
    n_pad = npc * N_CORES
    HC = 2 * ncls + 4

    xp = np.zeros((n_pad, f_in), dtype=np.float32)
    xp[lay["old2new"][: n_nodes]] = np.asarray(x, dtype=np.float32)

    wcat = np.concatenate(
        [W1, (W1 @ a1_src)[:, None], W2, (W2 @ a2_src)[:, None],
         (W1 @ a1_dst)[:, None], (W2 @ a2_dst)[:, None]],
        axis=1,
    ).astype(np.float32)
    assert wcat.shape == (hidden, HC)
    wd = np.stack([W1 @ a1_dst, W2 @ a2_dst], axis=1).astype(np.float32)
    bb = np.broadcast_to(
        np.concatenate([b1, b2])[None, :], (P, 2 * ncls)
    ).astype(np.float32).copy()

    in_maps = []
    for c in range(N_CORES):
        xc = xp[c * npc : (c + 1) * npc]
        m = {
            "xT": np.ascontiguousarray(xc.T),
            "xrow": np.ascontiguousarray(xc),
            "wmlp": np.asarray(W_mlp, dtype=np.float32),
            "bmlp": np.asarray(b_mlp, dtype=np.float32)[:, None].copy(),
            "wcat": wcat,
            "wd": wd,
            "bb": bb,
            "padm": lay["padm"],
            "gidx": np.ascontiguousarray(lay["gidx"][c]),
            "mgidx": np.ascontiguousarray(lay["mgidx"][c]),
        }
        for q in range(N_WIN):
            xq = xc[lay["node_at"][c, q]]
            m[f"xTq{q}"] = np.ascontiguousarray(xq.T)
        in_maps.append(m)

    res = bass_utils.run_bass_kernel_spmd(
        nc, in_maps, core_ids=list(range(N_CORES)), trace=trace
    )
    outs = np.concatenate([r["out"] for r in res.results], axis=0)
    final = outs[lay["old2new"][: n_nodes]]
    return final, res


def kernel(x, edge_index, W_mlp, b_mlp, W1, a1_src, a1_dst, b1,
           W2, a2_src, a2_dst, b2, trace=False, _ret_res=False):
    x = np.asarray(x)
    lay = _build_layout(edge_index, x.shape[0])
    nc = _build_program(lay, x.shape[1], W_mlp.shape[1], W1.shape[1])
    out, res = _run(nc, lay, x, W_mlp, b_mlp, W1, a1_src, a1_dst, b1,
                    W2, a2_src, a2_dst, b2, trace=trace)
    if _ret_res:
        return out, res
    return out


# revision 14
# speedup vs baseline: 1.1601x; 1.1601x over previous
"""GAT-style 2-conv GNN forward on 8 Trainium2 NeuronCores.

Strategy (graph/data parallel): nodes partitioned across 8 cores by
destination range; per-edge source-row gathers via InstDMAGatherAnt (int16
indices) against a replicated node-feature table.

Perf notes vs. the first working version (3.14 ms):
  - dma_gather costs ~7.8 ns per index of serialized GPSIMD time; total
    gathered indices are the kernel's critical resource.
  - d-values (per-dst attention bias) are RECOMPUTED per window order from
    host-permuted x on the idle TensorEngine instead of being gathered
    (-50k indices/core).
  - Windows are local row-quarters (not core pairs), so the table AllGather
    splits into 4 sub-collectives pipelined against window processing.
  - Pad rows are distributed per-quarter so each window table has dummy
    rows for grid padding.
"""

import sys

sys.path.insert(0, "/opt/trn_rl_repo")

import contextlib

import numpy as np

import concourse.bacc as bacc
import concourse.bass as bass
import concourse.bass_utils as bass_utils
import concourse.mybir as mybir
import concourse.tile as tile
from concourse import library_config
from concourse.masks import make_identity

FP32 = mybir.dt.float32
FP16 = mybir.dt.float16
INT16 = mybir.dt.int16

N_CORES = 8
N_WIN = 4
P = 128
ELEM = 128  # fp16 cols per table row = 256B

# table row columns
S1C, S2C, D1C, D2C = 32, 65, 66, 67
H1B, H2B = 0, 33
# partial row columns
PN1, PN2, PD1, PD2 = 0, 32, 64, 65
NEG_SLOPE = 0.2
DUMMY_S = -30000.0

S_CHUNK = 96  # grid columns per chunk (12288 idxs: descriptor-ring limit)
QTILES = (25, 25, 24, 24)  # tiles per window-quarter (sum = 98)


def _wrap16(stream):
    """[n] -> [128, n//16] int16 in the 16-partition wrapped+replicated layout."""
    n = stream.shape[0]
    assert n % 16 == 0
    w = stream.reshape(n // 16, 16).T.astype(np.int16)  # [16, n//16]
    return np.tile(w, (8, 1))


def _build_layout(edge_index, n_nodes):
    src = np.asarray(edge_index[0], dtype=np.int64)
    dst = np.asarray(edge_index[1], dtype=np.int64)
    E = src.shape[0]

    npc_raw = -(-n_nodes // N_CORES)
    tiles = -(-npc_raw // P)
    npc = tiles * P
    if npc == npc_raw:  # ensure pad rows exist
        tiles += 1
        npc += P
    n_pads = npc - npc_raw
    assert tiles == sum(QTILES)

    # quarter row layout: each quarter gets some pad rows at its end
    qrows = np.array(QTILES, dtype=np.int64) * P          # rows per quarter
    qrow_start = np.concatenate([[0], np.cumsum(qrows)])  # [5]
    padq = np.full(N_WIN, n_pads // N_WIN, dtype=np.int64)
    padq[: n_pads % N_WIN] += 1
    qreal = qrows - padq                                   # real rows/quarter
    qreal_start = np.concatenate([[0], np.cumsum(qreal)])  # over raw index
    assert qreal_start[-1] == npc_raw

    wsize = (qrows * N_CORES).astype(np.int64)             # table rows/window
    assert (wsize <= 32768).all()
    pad_rel = qreal.copy()  # core-0 pad start, window-relative

    # old local offset -> new local row
    off = np.arange(npc_raw)
    oq = np.searchsorted(qreal_start[1:], off, side="right")
    new_local = qrow_start[oq] + (off - qreal_start[oq])

    old2new = np.empty(n_nodes, dtype=np.int64)
    for c in range(N_CORES):
        lo = c * npc_raw
        hi = min(lo + npc_raw, n_nodes)
        old2new[lo:hi] = c * npc + new_local[: hi - lo]

    new_src = old2new[src]
    new_dst = old2new[dst]
    dst_core = new_dst // npc
    dst_local = new_dst % npc
    src_core = new_src // npc
    src_local = new_src % npc
    src_win = np.searchsorted(qrow_start[1:4], src_local, side="right")
    # window-table index of each edge's source
    src_tab = src_core * qrows[src_win] + (src_local - qrow_start[src_win])

    # per (core, window) in-degree
    qdeg = np.zeros((N_CORES, N_WIN, npc), dtype=np.int64)
    np.add.at(qdeg, (dst_core, src_win, dst_local), 1)

    node_at = np.empty((N_CORES, N_WIN, npc), dtype=np.int64)
    pos_of = np.empty((N_CORES, N_WIN, npc), dtype=np.int64)
    for c in range(N_CORES):
        for q in range(N_WIN):
            o = np.argsort(-qdeg[c, q], kind="stable")
            node_at[c, q] = o
            pos_of[c, q, o] = np.arange(npc)

    # shared tile degree profile per window
    D_q = np.zeros((N_WIN, tiles), dtype=np.int64)
    for q in range(N_WIN):
        sorted_deg = np.take_along_axis(qdeg[:, q, :], node_at[:, q, :], axis=1)
        D_q[q] = sorted_deg[:, ::P].max(axis=0)

    fb_q = np.zeros((N_WIN, tiles + 1), dtype=np.int64)
    for q in range(N_WIN):
        fb_q[q, 1:] = np.cumsum(D_q[q])
    slots_q = fb_q[:, -1].copy()

    # edge -> grid cell
    pos = pos_of[dst_core, src_win, dst_local]
    t = pos // P
    p = pos % P
    key = (dst_core * N_WIN + src_win) * npc + dst_local
    order = np.argsort(key, kind="stable")
    sk = key[order]
    first = np.flatnonzero(np.r_[True, sk[1:] != sk[:-1]])
    group_start = np.repeat(first, np.diff(np.r_[first, E]))
    j = np.empty(E, dtype=np.int64)
    j[order] = np.arange(E) - group_start
    assert (j < D_q[src_win, t]).all()
    col = fb_q[src_win, t] + j

    gidx = np.empty((N_CORES, P, int(slots_q.sum()) * 8), dtype=np.int16)
    wbase = np.concatenate([[0], np.cumsum(slots_q)])
    for c in range(N_CORES):
        for q in range(N_WIN):
            sq = int(slots_q[q])
            stream = np.full(sq * P, pad_rel[q], dtype=np.int64)
            m = (dst_core == c) & (src_win == q)
            stream[col[m] * P + p[m]] = src_tab[m]
            assert stream.max() < wsize[q] and stream.min() >= 0
            gidx[c, :, int(wbase[q]) * 8 : int(wbase[q] + sq) * 8] = _wrap16(
                stream
            )

    # merge-gather index streams (partial q-order -> common order)
    mgidx = np.empty((N_CORES, P, N_WIN * npc // 16), dtype=np.int16)
    for c in range(N_CORES):
        for q in range(N_WIN):
            sl = slice(q * npc // 16, (q + 1) * npc // 16)
            mgidx[c, :, sl] = _wrap16(pos_of[c, q])

    # chunk structure per window
    win_chunks = []
    for q in range(N_WIN):
        runs = []
        t0 = 0
        for tt in range(1, tiles + 1):
            if tt == tiles or D_q[q, tt] != D_q[q, t0]:
                if D_q[q, t0] > 0:
                    runs.append((t0, tt - t0, int(D_q[q, t0])))
                t0 = tt
        pieces = []
        for (rt0, g, d) in runs:
            max_g = max(1, S_CHUNK // d)
            s = 0
            while s < g:
                gg = min(max_g, g - s)
                pieces.append((rt0 + s, gg, d))
                s += gg
        chunks = []
        cur, cur_cols = [], 0
        for pc in pieces:
            need = pc[1] * pc[2]
            assert need <= S_CHUNK
            if cur_cols + need > S_CHUNK:
                chunks.append(cur)
                cur, cur_cols = [], 0
            cur.append(pc)
            cur_cols += need
        if cur:
            chunks.append(cur)
        win_chunks.append(chunks)

    # pad-row mask in common order [P, tiles]
    padm = np.zeros((npc,), dtype=np.float16)
    for q in range(N_WIN):
        padm[qrow_start[q] + qreal[q] : qrow_start[q + 1]] = DUMMY_S
    padm = np.ascontiguousarray(padm.reshape(tiles, P).T)

    return dict(
        npc_raw=npc_raw, npc=npc, tiles=tiles, wsize=wsize, qrows=qrows,
        qrow_start=qrow_start, old2new=old2new, D_q=D_q, fb_q=fb_q,
        slots_q=slots_q, gidx=gidx, mgidx=mgidx, win_chunks=win_chunks,
        node_at=node_at, padm=padm,
    )


def _build_program(lay, f_in, hidden, ncls):
    tiles = lay["tiles"]
    npc = lay["npc"]
    wsize = lay["wsize"]
    qrow_start = lay["qrow_start"]
    slots_q = lay["slots_q"]
    fb_q = lay["fb_q"]
    win_chunks = lay["win_chunks"]
    F = 2 * ncls + 1
    assert F == f_in
    HC = 2 * ncls + 4

    nc = bacc.Bacc("TRN2", target_bir_lowering=False, debug=False,
                   enable_asserts=False, num_devices=N_CORES,
                   num_swdge_queues=2)

    xT_in = nc.dram_tensor("xT", [f_in, npc], FP32, kind="ExternalInput").ap()
    xTq_in = [
        nc.dram_tensor(f"xTq{q}", [f_in, npc], FP32, kind="ExternalInput").ap()
        for q in range(N_WIN)
    ]
    x_in = nc.dram_tensor("xrow", [npc, f_in], FP32, kind="ExternalInput").ap()
    wmlp_in = nc.dram_tensor("wmlp", [f_in, hidden], FP32, kind="ExternalInput").ap()
    bmlp_in = nc.dram_tensor("bmlp", [hidden, 1], FP32, kind="ExternalInput").ap()
    wcat_in = nc.dram_tensor("wcat", [hidden, HC], FP32, kind="ExternalInput").ap()
    wd_in = nc.dram_tensor("wd", [hidden, 2], FP32, kind="ExternalInput").ap()
    bb_in = nc.dram_tensor("bb", [P, 2 * ncls], FP32, kind="ExternalInput").ap()
    padm_in = nc.dram_tensor("padm", [P, tiles], FP16, kind="ExternalInput").ap()
    gidx_in = nc.dram_tensor(
        "gidx", [P, int(slots_q.sum()) * 8], INT16, kind="ExternalInput"
    ).ap()
    mgidx_in = nc.dram_tensor(
        "mgidx", [P, N_WIN * npc // 16], INT16, kind="ExternalInput"
    ).ap()
    out_t = nc.dram_tensor("out", [npc, F], FP32, kind="ExternalOutput").ap()

    with tile.TileContext(nc) as tc:
        with contextlib.ExitStack() as ctx:
            persist = ctx.enter_context(tc.tile_pool(name="persist", bufs=1))
            dram = ctx.enter_context(tc.tile_pool(name="dram", bufs=1, space="DRAM"))
            cpool = ctx.enter_context(tc.tile_pool(name="consts", bufs=1))

            nc.gpsimd.load_library(library_config.mlp)

            x3buf = persist.tile([P, tiles], FP32)
            bb_sb = persist.tile([P, 2 * ncls], FP32)
            dq_sb = persist.tile([P, N_WIN, tiles, 2], FP32)
            nc.sync.dma_start(out=bb_sb[:], in_=bb_in[:])

            hloc_d = [
                dram.tile([int(lay["qrows"][q]), ELEM], FP16,
                          name=f"hloc{q}", tag=f"hloc{q}")
                for q in range(N_WIN)
            ]
            htab_d = [
                dram.tile([int(wsize[q]), ELEM], FP16, name=f"htab{q}",
                          tag=f"htab{q}")
                for q in range(N_WIN)
            ]
            part_d = [dram.tile([npc, ELEM], FP16, name=f"part{q}",
                                tag=f"part{q}")
                      for q in range(N_WIN)]

            wmlp_sb = cpool.tile([f_in, hidden], FP32)
            bmlp_sb = cpool.tile([hidden, 1], FP32)
            wd_sb = cpool.tile([hidden, 2], FP32)
            ident = cpool.tile([P, P], FP32)
            nc.sync.dma_start(out=wmlp_sb[:], in_=wmlp_in[:])
            nc.sync.dma_start(out=bmlp_sb[:], in_=bmlp_in[:])
            nc.sync.dma_start(out=wd_sb[:], in_=wd_in[:])
            make_identity(nc, ident[:])

            # Phase-3 pools created first so their SBUF ranges don't overlap
            # phase-1/2 tiles (address reuse adds a WAR dep that stalls the
            # first gather until all of phase 1 retires).
            ph3stack = contextlib.ExitStack()
            gip = ph3stack.enter_context(tc.tile_pool(name="gi", bufs=1))
            msgp = ph3stack.enter_context(tc.tile_pool(name="msg", bufs=2))
            scp = ph3stack.enter_context(tc.tile_pool(name="sc", bufs=2))
            exp_ = ph3stack.enter_context(tc.tile_pool(name="exb", bufs=2))
            ptp = ph3stack.enter_context(tc.tile_pool(name="pt", bufs=1))
            gidx_sb = gip.tile([P, int(slots_q.sum()) * 8], INT16)
            nc.sync.dma_start(out=gidx_sb[:], in_=gidx_in[:])

            # ---------------- Phase 1: dense local features ----------------
            with tc.tile_pool(name="ph1c", bufs=1) as c1pool, \
                 tc.tile_pool(name="ph1", bufs=3) as ph1, \
                 tc.tile_pool(name="ph1x", bufs=2) as ph1x, \
                 tc.tile_pool(name="hl", bufs=1) as hlp, \
                 tc.tile_pool(name="ps1", bufs=2, space="PSUM") as ps1, \
                 tc.tile_pool(name="ps2", bufs=2, space="PSUM") as ps2, \
                 tc.tile_pool(name="ps3", bufs=2, space="PSUM") as ps3:
                wcat_sb = c1pool.tile([hidden, HC], FP32)
                padm_sb = c1pool.tile([P, tiles], FP16)
                nc.sync.dma_start(out=wcat_sb[:], in_=wcat_in[:])
                nc.sync.dma_start(out=padm_sb[:], in_=padm_in[:])

                hq_sb = [
                    hlp.tile([P, QTILES[q], ELEM], FP16, name=f"hq{q}",
                             tag=f"hq{q}")
                    for q in range(N_WIN)
                ]
                for q in range(N_WIN):
                    nc.vector.memset(hq_sb[q][:], 0.0)

                XCH = 16
                q_next = 0
                for t in range(tiles):
                    if t % XCH == 0:
                        g = min(XCH, tiles - t)
                        xt_sb = ph1x.tile([f_in, XCH * P], FP32, tag="xt")
                        nc.sync.dma_start(
                            out=xt_sb[:, : g * P],
                            in_=xT_in[:, t * P : (t + g) * P],
                        )
                    xoff = (t % XCH) * P
                    psA = ps1.tile([P, P], FP32, space="PSUM")
                    nc.tensor.matmul(
                        out=psA[:], lhsT=wmlp_sb[:],
                        rhs=xt_sb[:, xoff : xoff + P],
                        start=True, stop=True,
                    )
                    x0t = ph1.tile([P, P], FP32, tag="x0t")
                    nc.scalar.activation(
                        out=x0t[:], in_=psA[:],
                        func=mybir.ActivationFunctionType.Relu,
                        bias=bmlp_sb[:, 0:1], scale=1.0,
                    )
                    psH = ps2.tile([P, HC], FP32, space="PSUM")
                    nc.tensor.matmul(
                        out=psH[:], lhsT=x0t[:], rhs=wcat_sb[:],
                        start=True, stop=True,
                    )
                    nc.vector.tensor_copy(
                        out=hq_sb[q_next][:, t - int(qrow_start[q_next] // P), 0:HC],
                        in_=psH[:],
                    )
                    psT = ps3.tile([P, P], FP32, space="PSUM")
                    nc.tensor.transpose(out=psT[:], in_=x0t[:], identity=ident[:])
                    nc.vector.tensor_reduce(
                        out=x3buf[:, t : t + 1], in_=psT[:],
                        axis=mybir.AxisListType.X, op=mybir.AluOpType.max,
                    )
                    # quarter complete -> mask pads, write to HBM
                    if t + 1 == qrow_start[q_next + 1] // P:
                        tb = int(qrow_start[q_next] // P)
                        te = t + 1
                        for scol in (S1C, S2C):
                            nc.vector.tensor_tensor(
                                out=hq_sb[q_next][:, :, scol : scol + 1],
                                in0=hq_sb[q_next][:, :, scol : scol + 1],
                                in1=padm_sb[:, tb:te].unsqueeze(2),
                                op=mybir.AluOpType.add,
                            )
                        nc.sync.dma_start(
                            out=hloc_d[q_next][:].rearrange(
                                "(t p) c -> p t c", p=P
                            ),
                            in_=hq_sb[q_next][:],
                        )
                        q_next += 1

            # ------------- Phase 2: recompute d per window order -----------
            with tc.tile_pool(name="ph2x", bufs=2) as ph2x, \
                 tc.tile_pool(name="ph2", bufs=3) as ph2, \
                 tc.tile_pool(name="psA2", bufs=2, space="PSUM") as psa2, \
                 tc.tile_pool(name="psD2", bufs=2, space="PSUM") as psd2:
                XCH = 16
                for q in range(N_WIN):
                    for t in range(tiles):
                        if t % XCH == 0:
                            g = min(XCH, tiles - t)
                            xq_sb = ph2x.tile([f_in, XCH * P], FP32, tag="xq")
                            nc.sync.dma_start(
                                out=xq_sb[:, : g * P],
                                in_=xTq_in[q][:, t * P : (t + g) * P],
                            )
                        xoff = (t % XCH) * P
                        psA = psa2.tile([P, P], FP32, space="PSUM")
                        nc.tensor.matmul(
                            out=psA[:], lhsT=wmlp_sb[:],
                            rhs=xq_sb[:, xoff : xoff + P],
                            start=True, stop=True,
                        )
                        x0q = ph2.tile([P, P], FP32, tag="x0q")
                        nc.scalar.activation(
                            out=x0q[:], in_=psA[:],
                            func=mybir.ActivationFunctionType.Relu,
                            bias=bmlp_sb[:, 0:1], scale=1.0,
                        )
                        psD = psd2.tile([P, 2], FP32, space="PSUM")
                        nc.tensor.matmul(
                            out=psD[:], lhsT=x0q[:], rhs=wd_sb[:],
                            start=True, stop=True,
                        )
                        nc.vector.tensor_copy(
                            out=dq_sb[:, q, t, :], in_=psD[:]
                        )

            # ---------------- Phase 3: per-window gather + reduce ----------
            if True:
                wbase = np.concatenate([[0], np.cumsum(slots_q)]).astype(int)

                def _trigger_ag(q):
                    nc.gpsimd.collective_compute(
                        "AllGather",
                        mybir.AluOpType.bypass,
                        replica_groups=[list(range(N_CORES))],
                        ins=[hloc_d[q][:].opt()],
                        outs=[htab_d[q][:].opt()],
                    )

                _trigger_ag(0)
                for q in range(N_WIN):
                    d1q = dq_sb[:, q, :, 0]
                    d2q = dq_sb[:, q, :, 1]

                    partial = ptp.tile([P, tiles, ELEM], FP16, tag="partial")
                    nc.vector.memset(partial[:], 0.0)

                    tab = htab_d[q][:]
                    for ci, chunk in enumerate(win_chunks[q]):
                        ccols = sum(g * d for (_, g, d) in chunk)
                        cb = int(fb_q[q, chunk[0][0]])
                        msg = msgp.tile([P, S_CHUNK, ELEM], FP16, tag="msg")
                        ib = (int(wbase[q]) + cb) * 8
                        nc.gpsimd.dma_gather(
                            out_ap=msg[:, :ccols, :],
                            in_ap=tab,
                            idxs_ap=gidx_sb[:, ib : ib + ccols * 8],
                            num_idxs=ccols * P,
                            num_idxs_reg=ccols * P,
                            elem_size=ELEM,
                            single_packet=False,
                            queue_num=ci % 2,
                        )
                        if ci == 0 and q + 1 < N_WIN:
                            _trigger_ag(q + 1)
                        for (rt0, g, d) in chunk:
                            s0 = int(fb_q[q, rt0]) - cb
                            mv = msg[:, s0 : s0 + g * d, :].rearrange(
                                "p (g e) c -> p g e c", e=d
                            )
                            for (hb, sc_, dq_, nb, db) in (
                                (H1B, S1C, d1q, PN1, PD1),
                                (H2B, S2C, d2q, PN2, PD2),
                            ):
                                exb = exp_.tile([P, S_CHUNK], FP32, tag="exb")
                                exv = exb[:, : g * d].rearrange(
                                    "p (g e) -> p g e", e=d
                                )
                                nc.vector.tensor_tensor(
                                    out=exv,
                                    in0=mv[:, :, :, sc_],
                                    in1=dq_[:, rt0 : rt0 + g]
                                    .unsqueeze(2)
                                    .broadcast_to([P, g, d]),
                                    op=mybir.AluOpType.add,
                                )
                                lrb = exp_.tile([P, S_CHUNK], FP32, tag="lrb")
                                lrv = lrb[:, : g * d].rearrange(
                                    "p (g e) -> p g e", e=d
                                )
                                nc.vector.tensor_scalar_mul(lrv, exv, NEG_SLOPE)
                                nc.vector.tensor_tensor(
                                    out=exv, in0=exv, in1=lrv,
                                    op=mybir.AluOpType.max,
                                )
                                nc.scalar.activation(
                                    out=exv, in_=exv,
                                    func=mybir.ActivationFunctionType.Exp,
                                )
                                sct = scp.tile(
                                    [P, S_CHUNK, ncls], FP32, tag="sc"
                                )
                                scv = sct[:, : g * d, :].rearrange(
                                    "p (g e) c -> p g e c", e=d
                                )
                                nc.vector.tensor_tensor(
                                    out=scv,
                                    in0=mv[:, :, :, hb : hb + ncls],
                                    in1=exv.unsqueeze(3).broadcast_to(
                                        [P, g, d, ncls]
                                    ),
                                    op=mybir.AluOpType.mult,
                                )
                                with nc.allow_low_precision("fp16 partials"):
                                    nc.vector.tensor_reduce(
                                        out=partial[:, rt0 : rt0 + g, nb : nb + ncls],
                                        in_=sct[:, : g * d, :].rearrange(
                                            "p (g e) c -> p g c e", e=d
                                        ),
                                        axis=mybir.AxisListType.X,
                                        op=mybir.AluOpType.add,
                                    )
                                    nc.vector.tensor_reduce(
                                        out=partial[:, rt0 : rt0 + g, db],
                                        in_=exv,
                                        axis=mybir.AxisListType.X,
                                        op=mybir.AluOpType.add,
                                    )
                    nc.sync.dma_start(
                        out=part_d[q][:].rearrange("(t p) c -> p t c", p=P),
                        in_=partial[:],
                    )

            ph3stack.close()

            # ---------------- Phase 3.9: merge partials --------------------
            with tc.tile_pool(name="mg", bufs=2) as mgp, \
                 tc.tile_pool(name="mgi", bufs=1) as mgip, \
                 tc.tile_pool(name="acc", bufs=1) as accp, \
                 tc.tile_pool(name="fin", bufs=1) as finp, \
                 tc.tile_pool(name="tmp", bufs=1) as tmpp:
                mgidx_sb = mgip.tile([P, N_WIN * npc // 16], INT16)
                nc.sync.dma_start(out=mgidx_sb[:], in_=mgidx_in[:])
                acc = accp.tile([P, tiles, 2 * ncls + 2], FP32)
                for q in range(N_WIN):
                    mg = mgp.tile([P, tiles, ELEM], FP16, tag="mg")
                    ib = q * npc // 16
                    nc.gpsimd.dma_gather(
                        out_ap=mg[:],
                        in_ap=part_d[q][:],
                        idxs_ap=mgidx_sb[:, ib : ib + npc // 16],
                        num_idxs=npc,
                        num_idxs_reg=npc,
                        elem_size=ELEM,
                        single_packet=False,
                        queue_num=q % 2,
                    )
                    if q == 0:
                        nc.vector.tensor_copy(
                            out=acc[:], in_=mg[:, :, 0 : 2 * ncls + 2]
                        )
                    else:
                        nc.vector.tensor_tensor(
                            out=acc[:], in0=acc[:],
                            in1=mg[:, :, 0 : 2 * ncls + 2],
                            op=mybir.AluOpType.add,
                        )

                # ------------- Phase 4: normalize + residual + lsm ---------
                xin = finp.tile([P, tiles, F], FP32)
                nc.sync.dma_start(
                    out=xin[:], in_=x_in[:].rearrange("(t p) f -> p t f", p=P)
                )
                rden = tmpp.tile([P, tiles], FP32, tag="rden")
                for conv in range(2):
                    numv = acc[:, :, conv * ncls : (conv + 1) * ncls]
                    denv = acc[:, :, 2 * ncls + conv]
                    nc.vector.tensor_scalar_add(denv, denv, 1e-16)
                    nc.vector.reciprocal(out=rden[:], in_=denv)
                    nc.vector.tensor_tensor(
                        out=numv, in0=numv,
                        in1=rden[:].unsqueeze(2).broadcast_to([P, tiles, ncls]),
                        op=mybir.AluOpType.mult,
                    )
                    nc.vector.tensor_tensor(
                        out=numv, in0=numv,
                        in1=bb_sb[:, conv * ncls : (conv + 1) * ncls]
                        .unsqueeze(1)
                        .broadcast_to([P, tiles, ncls]),
                        op=mybir.AluOpType.add,
                    )
                    if conv == 0:
                        nc.vector.tensor_scalar_max(numv, numv, 0.0)
                    nc.vector.tensor_tensor(
                        out=xin[:, :, conv * ncls : (conv + 1) * ncls],
                        in0=xin[:, :, conv * ncls : (conv + 1) * ncls],
                        in1=numv,
                        op=mybir.AluOpType.add,
                    )
                nc.vector.tensor_tensor(
                    out=xin[:, :, 2 * ncls], in0=xin[:, :, 2 * ncls],
                    in1=x3buf[:], op=mybir.AluOpType.add,
                )
                mx = tmpp.tile([P, tiles], FP32, tag="mx")
                nc.vector.tensor_reduce(
                    out=mx[:], in_=xin[:], axis=mybir.AxisListType.X,
                    op=mybir.AluOpType.max,
                )
                nc.vector.tensor_tensor(
                    out=xin[:], in0=xin[:],
                    in1=mx[:].unsqueeze(2).broadcast_to([P, tiles, F]),
                    op=mybir.AluOpType.subtract,
                )
                et = tmpp.tile([P, tiles, F], FP32, tag="et")
                nc.scalar.activation(
                    out=et[:], in_=xin[:],
                    func=mybir.ActivationFunctionType.Exp,
                )
                sm = tmpp.tile([P, tiles], FP32, tag="sm")
                nc.vector.tensor_reduce(
                    out=sm[:], in_=et[:], axis=mybir.AxisListType.X,
                    op=mybir.AluOpType.add,
                )
                lg = tmpp.tile([P, tiles], FP32, tag="lg")
                nc.scalar.activation(
                    out=lg[:], in_=sm[:],
                    func=mybir.ActivationFunctionType.Ln,
                )
                nc.vector.tensor_tensor(
                    out=xin[:], in0=xin[:],
                    in1=lg[:].unsqueeze(2).broadcast_to([P, tiles, F]),
                    op=mybir.AluOpType.subtract,
                )
                nc.sync.dma_start(
                    out=out_t[:].rearrange("(t p) f -> p t f", p=P), in_=xin[:]
                )

    nc.compile()
    return nc


def _run(nc, lay, x, W_mlp, b_mlp, W1, a1_src, a1_dst, b1,
         W2, a2_src, a2_dst, b2, trace=False):
    n_nodes, f_in = x.shape
    hidden = W_mlp.shape[1]
    ncls = W1.shape[1]
    npc = lay["npc"]
    n_pad = npc * N_CORES
    HC = 2 * ncls + 4

    xp = np.zeros((n_pad, f_in), dtype=np.float32)
    xp[lay["old2new"][: n_nodes]] = np.asarray(x, dtype=np.float32)

    wcat = np.concatenate(
        [W1, (W1 @ a1_src)[:, None], W2, (W2 @ a2_src)[:, None],
         (W1 @ a1_dst)[:, None], (W2 @ a2_dst)[:, None]],
        axis=1,
    ).astype(np.float32)
    assert wcat.shape == (hidden, HC)
    wd = np.stack([W1 @ a1_dst, W2 @ a2_dst], axis=1).astype(np.float32)
    bb = np.broadcast_to(
        np.concatenate([b1, b2])[None, :], (P, 2 * ncls)
    ).astype(np.float32).copy()

    in_maps = []
    for c in range(N_CORES):
        xc = xp[c * npc : (c + 1) * npc]
        m = {
            "xT": np.ascontiguousarray(xc.T),
            "xrow": np.ascontiguousarray(xc),
            "wmlp": np.asarray(W_mlp, dtype=np.float32),
            "bmlp": np.asarray(b_mlp, dtype=np.float32)[:, None].copy(),
            "wcat": wcat,
            "wd": wd,
            "bb": bb,
            "padm": lay["padm"],
            "gidx": np.ascontiguousarray(lay["gidx"][c]),
            "mgidx": np.ascontiguousarray(lay["mgidx"][c]),
        }
        for q in range(N_WIN):
            xq = xc[lay["node_at"][c, q]]
            m[f"xTq{q}"] = np.ascontiguousarray(xq.T)
        in_maps.append(m)

    res = bass_utils.run_bass_kernel_spmd(
        nc, in_maps, core_ids=list(range(N_CORES)), trace=trace
    )
    outs = np.concatenate([r["out"] for r in res.results], axis=0)
    final = outs[lay["old2new"][: n_nodes]]
    return final, res


def kernel(x, edge_index, W_mlp, b_mlp, W1, a1_src, a1_dst, b1,
           W2, a2_src, a2_dst, b2, trace=False, _ret_res=False):
    x = np.asarray(x)
    lay = _build_layout(edge_index, x.shape[0])
    nc = _build_program(lay, x.shape[1], W_mlp.shape[1], W1.shape[1])
    out, res = _run(nc, lay, x, W_mlp, b_mlp, W1, a1_src, a1_dst, b1,
                    W2, a2_src, a2_dst, b2, trace=trace)
    if _ret_res:
        return out, res
    return out


# revision 20
# speedup vs baseline: 1.2070x; 1.0404x over previous
"""GAT-style 2-conv GNN forward on 8 Trainium2 NeuronCores.

Strategy (graph/data parallel): nodes partitioned across 8 cores by
destination range; per-edge source-row gathers via InstDMAGatherAnt (int16
indices) against a replicated node-feature table.

Perf notes vs. the first working version (3.14 ms):
  - dma_gather costs ~7.8 ns per index of serialized GPSIMD time; total
    gathered indices are the kernel's critical resource.
  - d-values (per-dst attention bias) are RECOMPUTED per window order from
    host-permuted x on the idle TensorEngine instead of being gathered
    (-50k indices/core).
  - Windows are local row-quarters (not core pairs), so the table AllGather
    splits into 4 sub-collectives pipelined against window processing.
  - Pad rows are distributed per-quarter so each window table has dummy
    rows for grid padding.
"""

import sys

sys.path.insert(0, "/opt/trn_rl_repo")

import contextlib

import numpy as np

import concourse.bacc as bacc
import concourse.bass as bass
import concourse.bass_utils as bass_utils
import concourse.mybir as mybir
import concourse.tile as tile
from concourse import library_config
from concourse.masks import make_identity

FP32 = mybir.dt.float32
FP16 = mybir.dt.float16
INT16 = mybir.dt.int16

N_CORES = 8
N_WIN = 4
P = 128
ELEM = 128  # fp16 cols per table row = 256B

# table row columns
S1C, S2C, D1C, D2C = 32, 65, 66, 67
H1B, H2B = 0, 33
# partial row columns
PN1, PN2, PD1, PD2 = 0, 32, 64, 65
NEG_SLOPE = 0.2
DUMMY_S = -30000.0

S_CHUNK = 96  # grid columns per chunk (12288 idxs: descriptor-ring limit)
QTILES = (12, 29, 29, 28)  # window-quarters (sum=98); smaller first quarter
# so AG0 completes early and edge gathers start sooner


def _wrap16(stream):
    """[n] -> [128, n//16] int16 in the 16-partition wrapped+replicated layout."""
    n = stream.shape[0]
    assert n % 16 == 0
    w = stream.reshape(n // 16, 16).T.astype(np.int16)  # [16, n//16]
    return np.tile(w, (8, 1))


def _build_layout(edge_index, n_nodes):
    src = np.asarray(edge_index[0], dtype=np.int64)
    dst = np.asarray(edge_index[1], dtype=np.int64)
    E = src.shape[0]

    npc_raw = -(-n_nodes // N_CORES)
    tiles = -(-npc_raw // P)
    npc = tiles * P
    if npc == npc_raw:  # ensure pad rows exist
        tiles += 1
        npc += P
    n_pads = npc - npc_raw
    assert tiles == sum(QTILES)

    # quarter row layout: each quarter gets some pad rows at its end
    qrows = np.array(QTILES, dtype=np.int64) * P          # rows per quarter
    qrow_start = np.concatenate([[0], np.cumsum(qrows)])  # [5]
    padq = np.full(N_WIN, n_pads // N_WIN, dtype=np.int64)
    padq[: n_pads % N_WIN] += 1
    qreal = qrows - padq                                   # real rows/quarter
    qreal_start = np.concatenate([[0], np.cumsum(qreal)])  # over raw index
    assert qreal_start[-1] == npc_raw

    wsize = (qrows * N_CORES).astype(np.int64)             # table rows/window
    assert (wsize <= 32768).all()
    pad_rel = qreal.copy()  # core-0 pad start, window-relative

    # old local offset -> new local row
    off = np.arange(npc_raw)
    oq = np.searchsorted(qreal_start[1:], off, side="right")
    new_local = qrow_start[oq] + (off - qreal_start[oq])

    old2new = np.empty(n_nodes, dtype=np.int64)
    for c in range(N_CORES):
        lo = c * npc_raw
        hi = min(lo + npc_raw, n_nodes)
        old2new[lo:hi] = c * npc + new_local[: hi - lo]

    new_src = old2new[src]
    new_dst = old2new[dst]
    dst_core = new_dst // npc
    dst_local = new_dst % npc
    src_core = new_src // npc
    src_local = new_src % npc
    src_win = np.searchsorted(qrow_start[1:4], src_local, side="right")
    # window-table index of each edge's source
    src_tab = src_core * qrows[src_win] + (src_local - qrow_start[src_win])

    # per (core, window) in-degree
    qdeg = np.zeros((N_CORES, N_WIN, npc), dtype=np.int64)
    np.add.at(qdeg, (dst_core, src_win, dst_local), 1)

    node_at = np.empty((N_CORES, N_WIN, npc), dtype=np.int64)
    pos_of = np.empty((N_CORES, N_WIN, npc), dtype=np.int64)
    for c in range(N_CORES):
        for q in range(N_WIN):
            o = np.argsort(-qdeg[c, q], kind="stable")
            node_at[c, q] = o
            pos_of[c, q, o] = np.arange(npc)

    # shared tile degree profile per window
    D_q = np.zeros((N_WIN, tiles), dtype=np.int64)
    for q in range(N_WIN):
        sorted_deg = np.take_along_axis(qdeg[:, q, :], node_at[:, q, :], axis=1)
        D_q[q] = sorted_deg[:, ::P].max(axis=0)

    fb_q = np.zeros((N_WIN, tiles + 1), dtype=np.int64)
    for q in range(N_WIN):
        fb_q[q, 1:] = np.cumsum(D_q[q])
    slots_q = fb_q[:, -1].copy()

    # edge -> grid cell
    pos = pos_of[dst_core, src_win, dst_local]
    t = pos // P
    p = pos % P
    key = (dst_core * N_WIN + src_win) * npc + dst_local
    order = np.argsort(key, kind="stable")
    sk = key[order]
    first = np.flatnonzero(np.r_[True, sk[1:] != sk[:-1]])
    group_start = np.repeat(first, np.diff(np.r_[first, E]))
    j = np.empty(E, dtype=np.int64)
    j[order] = np.arange(E) - group_start
    assert (j < D_q[src_win, t]).all()
    col = fb_q[src_win, t] + j

    gidx = np.empty((N_CORES, P, int(slots_q.sum()) * 8), dtype=np.int16)
    wbase = np.concatenate([[0], np.cumsum(slots_q)])
    for c in range(N_CORES):
        for q in range(N_WIN):
            sq = int(slots_q[q])
            stream = np.full(sq * P, pad_rel[q], dtype=np.int64)
            m = (dst_core == c) & (src_win == q)
            stream[col[m] * P + p[m]] = src_tab[m]
            assert stream.max() < wsize[q] and stream.min() >= 0
            gidx[c, :, int(wbase[q]) * 8 : int(wbase[q] + sq) * 8] = _wrap16(
                stream
            )

    # merge-gather index streams (partial q-order -> common order)
    mgidx = np.empty((N_CORES, P, N_WIN * npc // 16), dtype=np.int16)
    for c in range(N_CORES):
        for q in range(N_WIN):
            sl = slice(q * npc // 16, (q + 1) * npc // 16)
            mgidx[c, :, sl] = _wrap16(pos_of[c, q])

    # chunk structure per window
    win_chunks = []
    for q in range(N_WIN):
        runs = []
        t0 = 0
        for tt in range(1, tiles + 1):
            if tt == tiles or D_q[q, tt] != D_q[q, t0]:
                if D_q[q, t0] > 0:
                    runs.append((t0, tt - t0, int(D_q[q, t0])))
                t0 = tt
        pieces = []
        for (rt0, g, d) in runs:
            max_g = max(1, S_CHUNK // d)
            s = 0
            while s < g:
                gg = min(max_g, g - s)
                pieces.append((rt0 + s, gg, d))
                s += gg
        chunks = []
        cur, cur_cols = [], 0
        for pc in pieces:
            need = pc[1] * pc[2]
            assert need <= S_CHUNK
            if cur_cols + need > S_CHUNK:
                chunks.append(cur)
                cur, cur_cols = [], 0
            cur.append(pc)
            cur_cols += need
        if cur:
            chunks.append(cur)
        win_chunks.append(chunks)

    # pad-row mask in common order [P, tiles]
    padm = np.zeros((npc,), dtype=np.float16)
    for q in range(N_WIN):
        padm[qrow_start[q] + qreal[q] : qrow_start[q + 1]] = DUMMY_S
    padm = np.ascontiguousarray(padm.reshape(tiles, P).T)

    return dict(
        npc_raw=npc_raw, npc=npc, tiles=tiles, wsize=wsize, qrows=qrows,
        qrow_start=qrow_start, old2new=old2new, D_q=D_q, fb_q=fb_q,
        slots_q=slots_q, gidx=gidx, mgidx=mgidx, win_chunks=win_chunks,
        node_at=node_at, padm=padm,
    )


def _build_program(lay, f_in, hidden, ncls):
    tiles = lay["tiles"]
    npc = lay["npc"]
    wsize = lay["wsize"]
    qrow_start = lay["qrow_start"]
    slots_q = lay["slots_q"]
    fb_q = lay["fb_q"]
    win_chunks = lay["win_chunks"]
    F = 2 * ncls + 1
    assert F == f_in
    HC = 2 * ncls + 4

    nc = bacc.Bacc("TRN2", target_bir_lowering=False, debug=False,
                   enable_asserts=False, num_devices=N_CORES,
                   num_swdge_queues=2)

    xT_in = nc.dram_tensor("xT", [f_in, npc], FP32, kind="ExternalInput").ap()
    xTq_in = [
        nc.dram_tensor(f"xTq{q}", [f_in, npc], FP32, kind="ExternalInput").ap()
        for q in range(N_WIN)
    ]
    x_in = nc.dram_tensor("xrow", [npc, f_in], FP32, kind="ExternalInput").ap()
    wmlp_in = nc.dram_tensor("wmlp", [f_in, hidden], FP32, kind="ExternalInput").ap()
    bmlp_in = nc.dram_tensor("bmlp", [hidden, 1], FP32, kind="ExternalInput").ap()
    wcat_in = nc.dram_tensor("wcat", [hidden, HC], FP32, kind="ExternalInput").ap()
    wd_in = nc.dram_tensor("wd", [hidden, 2], FP32, kind="ExternalInput").ap()
    bb_in = nc.dram_tensor("bb", [P, 2 * ncls], FP32, kind="ExternalInput").ap()
    padm_in = nc.dram_tensor("padm", [P, tiles], FP16, kind="ExternalInput").ap()
    gidx_in = nc.dram_tensor(
        "gidx", [P, int(slots_q.sum()) * 8], INT16, kind="ExternalInput"
    ).ap()
    mgidx_in = nc.dram_tensor(
        "mgidx", [P, N_WIN * npc // 16], INT16, kind="ExternalInput"
    ).ap()
    out_t = nc.dram_tensor("out", [npc, F], FP32, kind="ExternalOutput").ap()

    with tile.TileContext(nc) as tc:
        with contextlib.ExitStack() as ctx:
            persist = ctx.enter_context(tc.tile_pool(name="persist", bufs=1))
            dram = ctx.enter_context(tc.tile_pool(name="dram", bufs=1, space="DRAM"))
            cpool = ctx.enter_context(tc.tile_pool(name="consts", bufs=1))

            nc.gpsimd.load_library(library_config.mlp)

            x3buf = persist.tile([P, tiles], FP32)
            bb_sb = persist.tile([P, 2 * ncls], FP32)
            dq_sb = persist.tile([P, N_WIN, tiles, 2], FP32)
            nc.sync.dma_start(out=bb_sb[:], in_=bb_in[:])

            hloc_d = [
                dram.tile([int(lay["qrows"][q]), ELEM], FP16,
                          name=f"hloc{q}", tag=f"hloc{q}")
                for q in range(N_WIN)
            ]
            htab_d = [
                dram.tile([int(wsize[q]), ELEM], FP16, name=f"htab{q}",
                          tag=f"htab{q}")
                for q in range(N_WIN)
            ]
            part_d = [dram.tile([npc, ELEM], FP16, name=f"part{q}",
                                tag=f"part{q}")
                      for q in range(N_WIN)]

            wmlp_sb = cpool.tile([f_in, hidden], FP32)
            bmlp_sb = cpool.tile([hidden, 1], FP32)
            wd_sb = cpool.tile([hidden, 2], FP32)
            ident = cpool.tile([P, P], FP32)
            nc.sync.dma_start(out=wmlp_sb[:], in_=wmlp_in[:])
            nc.sync.dma_start(out=bmlp_sb[:], in_=bmlp_in[:])
            nc.sync.dma_start(out=wd_sb[:], in_=wd_in[:])
            make_identity(nc, ident[:])

            # Phase-3 pools created first so their SBUF ranges don't overlap
            # phase-1/2 tiles (address reuse adds a WAR dep that stalls the
            # first gather until all of phase 1 retires).
            ph3stack = contextlib.ExitStack()
            gip = ph3stack.enter_context(tc.tile_pool(name="gi", bufs=1))
            msgp = ph3stack.enter_context(tc.tile_pool(name="msg", bufs=3))
            scp = ph3stack.enter_context(tc.tile_pool(name="sc", bufs=2))
            exp_ = ph3stack.enter_context(tc.tile_pool(name="exb", bufs=2))
            ptp = ph3stack.enter_context(tc.tile_pool(name="pt", bufs=2))
            gidx_sb = gip.tile([P, int(slots_q.sum()) * 8], INT16)
            nc.sync.dma_start(out=gidx_sb[:], in_=gidx_in[:])

            # ---------------- Phase 1: dense local features ----------------
            with tc.tile_pool(name="ph1c", bufs=1) as c1pool, \
                 tc.tile_pool(name="ph1", bufs=3) as ph1, \
                 tc.tile_pool(name="ph1x", bufs=2) as ph1x, \
                 tc.tile_pool(name="hl", bufs=1) as hlp, \
                 tc.tile_pool(name="ps1", bufs=2, space="PSUM") as ps1, \
                 tc.tile_pool(name="ps2", bufs=2, space="PSUM") as ps2, \
                 tc.tile_pool(name="ps3", bufs=2, space="PSUM") as ps3:
                wcat_sb = c1pool.tile([hidden, HC], FP32)
                padm_sb = c1pool.tile([P, tiles], FP16)
                nc.sync.dma_start(out=wcat_sb[:], in_=wcat_in[:])
                nc.sync.dma_start(out=padm_sb[:], in_=padm_in[:])

                hq_sb = [
                    hlp.tile([P, max(QTILES), ELEM], FP16, name=f"hq{q}",
                             tag=f"hq{q % 2}", bufs=1)[:, : QTILES[q], :]
                    for q in range(N_WIN)
                ]
                for q in range(N_WIN):
                    nc.vector.memset(hq_sb[q][:], 0.0)

                XCH = 8
                q_next = 0
                for t in range(tiles):
                    if t % XCH == 0:
                        g = min(XCH, tiles - t)
                        xt_sb = ph1x.tile([f_in, XCH * P], FP32, tag="xt")
                        nc.sync.dma_start(
                            out=xt_sb[:, : g * P],
                            in_=xT_in[:, t * P : (t + g) * P],
                        )
                    xoff = (t % XCH) * P
                    psA = ps1.tile([P, P], FP32, space="PSUM")
                    nc.tensor.matmul(
                        out=psA[:], lhsT=wmlp_sb[:],
                        rhs=xt_sb[:, xoff : xoff + P],
                        start=True, stop=True,
                    )
                    x0t = ph1.tile([P, P], FP32, tag="x0t")
                    nc.scalar.activation(
                        out=x0t[:], in_=psA[:],
                        func=mybir.ActivationFunctionType.Relu,
                        bias=bmlp_sb[:, 0:1], scale=1.0,
                    )
                    psH = ps2.tile([P, HC], FP32, space="PSUM")
                    nc.tensor.matmul(
                        out=psH[:], lhsT=x0t[:], rhs=wcat_sb[:],
                        start=True, stop=True,
                    )
                    nc.vector.tensor_copy(
                        out=hq_sb[q_next][:, t - int(qrow_start[q_next] // P), 0:HC],
                        in_=psH[:],
                    )
                    psT = ps3.tile([P, P], FP32, space="PSUM")
                    nc.tensor.transpose(out=psT[:], in_=x0t[:], identity=ident[:])
                    nc.vector.tensor_reduce(
                        out=x3buf[:, t : t + 1], in_=psT[:],
                        axis=mybir.AxisListType.X, op=mybir.AluOpType.max,
                    )
                    # quarter complete -> mask pads, write to HBM
                    if t + 1 == qrow_start[q_next + 1] // P:
                        tb = int(qrow_start[q_next] // P)
                        te = t + 1
                        for scol in (S1C, S2C):
                            nc.vector.tensor_tensor(
                                out=hq_sb[q_next][:, :, scol : scol + 1],
                                in0=hq_sb[q_next][:, :, scol : scol + 1],
                                in1=padm_sb[:, tb:te].unsqueeze(2),
                                op=mybir.AluOpType.add,
                            )
                        nc.sync.dma_start(
                            out=hloc_d[q_next][:].rearrange(
                                "(t p) c -> p t c", p=P
                            ),
                            in_=hq_sb[q_next][:],
                        )
                        q_next += 1

            # ------------- Phase 2: recompute d per window order -----------
            with tc.tile_pool(name="ph2x", bufs=2) as ph2x, \
                 tc.tile_pool(name="ph2", bufs=3) as ph2, \
                 tc.tile_pool(name="psA2", bufs=2, space="PSUM") as psa2, \
                 tc.tile_pool(name="psD2", bufs=2, space="PSUM") as psd2:
                XCH = 16
                for q in range(N_WIN):
                    for t in range(tiles):
                        if t % XCH == 0:
                            g = min(XCH, tiles - t)
                            xq_sb = ph2x.tile([f_in, XCH * P], FP32, tag="xq")
                            nc.sync.dma_start(
                                out=xq_sb[:, : g * P],
                                in_=xTq_in[q][:, t * P : (t + g) * P],
                            )
                        xoff = (t % XCH) * P
                        psA = psa2.tile([P, P], FP32, space="PSUM")
                        nc.tensor.matmul(
                            out=psA[:], lhsT=wmlp_sb[:],
                            rhs=xq_sb[:, xoff : xoff + P],
                            start=True, stop=True,
                        )
                        x0q = ph2.tile([P, P], FP32, tag="x0q")
                        nc.scalar.activation(
                            out=x0q[:], in_=psA[:],
                            func=mybir.ActivationFunctionType.Relu,
                            bias=bmlp_sb[:, 0:1], scale=1.0,
                        )
                        psD = psd2.tile([P, 2], FP32, space="PSUM")
                        nc.tensor.matmul(
                            out=psD[:], lhsT=x0q[:], rhs=wd_sb[:],
                            start=True, stop=True,
                        )
                        nc.vector.tensor_copy(
                            out=dq_sb[:, q, t, :], in_=psD[:]
                        )

            # ---------------- Phase 3: per-window gather + reduce ----------
            if True:
                wbase = np.concatenate([[0], np.cumsum(slots_q)]).astype(int)

                def _trigger_ag(q):
                    nc.gpsimd.collective_compute(
                        "AllGather",
                        mybir.AluOpType.bypass,
                        replica_groups=[list(range(N_CORES))],
                        ins=[hloc_d[q][:].opt()],
                        outs=[htab_d[q][:].opt()],
                    )

                _trigger_ag(0)
                for q in range(N_WIN):
                    d1q = dq_sb[:, q, :, 0]
                    d2q = dq_sb[:, q, :, 1]

                    partial = ptp.tile([P, tiles, ELEM], FP16, tag="partial")
                    nc.vector.memset(partial[:], 0.0)

                    tab = htab_d[q][:]
                    for ci, chunk in enumerate(win_chunks[q]):
                        ccols = sum(g * d for (_, g, d) in chunk)
                        cb = int(fb_q[q, chunk[0][0]])
                        msg = msgp.tile([P, S_CHUNK, ELEM], FP16, tag="msg")
                        ib = (int(wbase[q]) + cb) * 8
                        nc.gpsimd.dma_gather(
                            out_ap=msg[:, :ccols, :],
                            in_ap=tab,
                            idxs_ap=gidx_sb[:, ib : ib + ccols * 8],
                            num_idxs=ccols * P,
                            num_idxs_reg=ccols * P,
                            elem_size=ELEM,
                            single_packet=False,
                            queue_num=ci % 2,
                        )
                        if ci == 0 and q + 1 < N_WIN:
                            _trigger_ag(q + 1)
                        for (rt0, g, d) in chunk:
                            s0 = int(fb_q[q, rt0]) - cb
                            mv = msg[:, s0 : s0 + g * d, :].rearrange(
                                "p (g e) c -> p g e c", e=d
                            )
                            for (hb, sc_, dq_, nb, db) in (
                                (H1B, S1C, d1q, PN1, PD1),
                                (H2B, S2C, d2q, PN2, PD2),
                            ):
                                exb = exp_.tile([P, S_CHUNK], FP16, tag="exb")
                                exv = exb[:, : g * d].rearrange(
                                    "p (g e) -> p g e", e=d
                                )
                                nc.vector.tensor_tensor(
                                    out=exv,
                                    in0=mv[:, :, :, sc_],
                                    in1=dq_[:, rt0 : rt0 + g]
                                    .unsqueeze(2)
                                    .broadcast_to([P, g, d]),
                                    op=mybir.AluOpType.add,
                                )
                                lrb = exp_.tile([P, S_CHUNK], FP16, tag="lrb")
                                lrv = lrb[:, : g * d].rearrange(
                                    "p (g e) -> p g e", e=d
                                )
                                nc.vector.tensor_scalar_mul(lrv, exv, NEG_SLOPE)
                                nc.vector.tensor_tensor(
                                    out=exv, in0=exv, in1=lrv,
                                    op=mybir.AluOpType.max,
                                )
                                nc.scalar.activation(
                                    out=exv, in_=exv,
                                    func=mybir.ActivationFunctionType.Exp,
                                )
                                sct = scp.tile(
                                    [P, S_CHUNK, ncls], FP16, tag="sc"
                                )
                                scv = sct[:, : g * d, :].rearrange(
                                    "p (g e) c -> p g e c", e=d
                                )
                                nc.vector.tensor_tensor(
                                    out=scv,
                                    in0=mv[:, :, :, hb : hb + ncls],
                                    in1=exv.unsqueeze(3).broadcast_to(
                                        [P, g, d, ncls]
                                    ),
                                    op=mybir.AluOpType.mult,
                                )
                                with nc.allow_low_precision("fp16 partials"):
                                    nc.vector.tensor_reduce(
                                        out=partial[:, rt0 : rt0 + g, nb : nb + ncls],
                                        in_=sct[:, : g * d, :].rearrange(
                                            "p (g e) c -> p g c e", e=d
                                        ),
                                        axis=mybir.AxisListType.X,
                                        op=mybir.AluOpType.add,
                                    )
                                    nc.vector.tensor_reduce(
                                        out=partial[:, rt0 : rt0 + g, db],
                                        in_=exv,
                                        axis=mybir.AxisListType.X,
                                        op=mybir.AluOpType.add,
                                    )
                    nc.sync.dma_start(
                        out=part_d[q][:].rearrange("(t p) c -> p t c", p=P),
                        in_=partial[:],
                    )

            ph3stack.close()

            # ---------------- Phase 3.9: merge partials --------------------
            with tc.tile_pool(name="mg", bufs=2) as mgp, \
                 tc.tile_pool(name="mgi", bufs=1) as mgip, \
                 tc.tile_pool(name="acc", bufs=1) as accp, \
                 tc.tile_pool(name="fin", bufs=1) as finp, \
                 tc.tile_pool(name="tmp", bufs=1) as tmpp:
                mgidx_sb = mgip.tile([P, N_WIN * npc // 16], INT16)
                nc.sync.dma_start(out=mgidx_sb[:], in_=mgidx_in[:])
                acc = accp.tile([P, tiles, 2 * ncls + 2], FP32)
                for q in range(N_WIN):
                    mg = mgp.tile([P, tiles, ELEM], FP16, tag="mg")
                    ib = q * npc // 16
                    nc.gpsimd.dma_gather(
                        out_ap=mg[:],
                        in_ap=part_d[q][:],
                        idxs_ap=mgidx_sb[:, ib : ib + npc // 16],
                        num_idxs=npc,
                        num_idxs_reg=npc,
                        elem_size=ELEM,
                        single_packet=False,
                        queue_num=q % 2,
                    )
                    if q == 0:
                        nc.vector.tensor_copy(
                            out=acc[:], in_=mg[:, :, 0 : 2 * ncls + 2]
                        )
                    else:
                        nc.vector.tensor_tensor(
                            out=acc[:], in0=acc[:],
                            in1=mg[:, :, 0 : 2 * ncls + 2],
                            op=mybir.AluOpType.add,
                        )

                # ------------- Phase 4: normalize + residual + lsm ---------
                xin = finp.tile([P, tiles, F], FP32)
                nc.sync.dma_start(
                    out=xin[:], in_=x_in[:].rearrange("(t p) f -> p t f", p=P)
                )
                rden = tmpp.tile([P, tiles], FP32, tag="rden")
                for conv in range(2):
                    numv = acc[:, :, conv * ncls : (conv + 1) * ncls]
                    denv = acc[:, :, 2 * ncls + conv]
                    nc.vector.tensor_scalar_add(denv, denv, 1e-16)
                    nc.vector.reciprocal(out=rden[:], in_=denv)
                    nc.vector.tensor_tensor(
                        out=numv, in0=numv,
                        in1=rden[:].unsqueeze(2).broadcast_to([P, tiles, ncls]),
                        op=mybir.AluOpType.mult,
                    )
                    nc.vector.tensor_tensor(
                        out=numv, in0=numv,
                        in1=bb_sb[:, conv * ncls : (conv + 1) * ncls]
                        .unsqueeze(1)
                        .broadcast_to([P, tiles, ncls]),
                        op=mybir.AluOpType.add,
                    )
                    if conv == 0:
                        nc.vector.tensor_scalar_max(numv, numv, 0.0)
                    nc.vector.tensor_tensor(
                        out=xin[:, :, conv * ncls : (conv + 1) * ncls],
                        in0=xin[:, :, conv * ncls : (conv + 1) * ncls],
                        in1=numv,
                        op=mybir.AluOpType.add,
                    )
                nc.vector.tensor_tensor(
                    out=xin[:, :, 2 * ncls], in0=xin[:, :, 2 * ncls],
                    in1=x3buf[:], op=mybir.AluOpType.add,
                )
                mx = tmpp.tile([P, tiles], FP32, tag="mx")
                nc.vector.tensor_reduce(
                    out=mx[:], in_=xin[:], axis=mybir.AxisListType.X,
                    op=mybir.AluOpType.max,
                )
                nc.vector.tensor_tensor(
                    out=xin[:], in0=xin[:],
                    in1=mx[:].unsqueeze(2).broadcast_to([P, tiles, F]),
                    op=mybir.AluOpType.subtract,
                )
                et = tmpp.tile([P, tiles, F], FP32, tag="et")
                nc.scalar.activation(
                    out=et[:], in_=xin[:],
                    func=mybir.ActivationFunctionType.Exp,
                )
                sm = tmpp.tile([P, tiles], FP32, tag="sm")
                nc.vector.tensor_reduce(
                    out=sm[:], in_=et[:], axis=mybir.AxisListType.X,
                    op=mybir.AluOpType.add,
                )
                lg = tmpp.tile([P, tiles], FP32, tag="lg")
                nc.scalar.activation(
                    out=lg[:], in_=sm[:],
                    func=mybir.ActivationFunctionType.Ln,
                )
                nc.vector.tensor_tensor(
                    out=xin[:], in0=xin[:],
                    in1=lg[:].unsqueeze(2).broadcast_to([P, tiles, F]),
                    op=mybir.AluOpType.subtract,
                )
                nc.sync.dma_start(
                    out=out_t[:].rearrange("(t p) f -> p t f", p=P), in_=xin[:]
                )

    nc.compile()
    return nc


def _run(nc, lay, x, W_mlp, b_mlp, W1, a1_src, a1_dst, b1,
         W2, a2_src, a2_dst, b2, trace=False):
    n_nodes, f_in = x.shape
    hidden = W_mlp.shape[1]
    ncls = W1.shape[1]
    npc = lay["npc"]
    n_pad = npc * N_CORES
    HC = 2 * ncls + 4

    xp = np.zeros((n_pad, f_in), dtype=np.float32)
    xp[lay["old2new"][: n_nodes]] = np.asarray(x, dtype=np.float32)

    wcat = np.concatenate(
        [W1, (W1 @ a1_src)[:, None], W2, (W2 @ a2_src)[:, None],
         (W1 @ a1_dst)[:, None], (W2 @ a2_dst)[:, None]],
        axis=1,
    ).astype(np.float32)
    assert wcat.shape == (hidden, HC)
    wd = np.stack([W1 @ a1_dst, W2 @ a2_dst], axis=1).astype(np.float32)
    bb = np.broadcast_to(
        np.concatenate([b1, b2])[None, :], (P, 2 * ncls)
    ).astype(np.float32).copy()

    in_maps = []
    for c in range(N_CORES):
        xc = xp[c * npc : (c + 1) * npc]
        m = {
            "xT": np.ascontiguousarray(xc.T),
            "xrow": np.ascontiguousarray(xc),
            "wmlp": np.asarray(W_mlp, dtype=np.float32),
            "bmlp": np.asarray(b_mlp, dtype=np.float32)[:, None].copy(),
            "wcat": wcat,
            "wd": wd,
            "bb": bb,
            "padm": lay["padm"],
            "gidx": np.ascontiguousarray(lay["gidx"][c]),
            "mgidx": np.ascontiguousarray(lay["mgidx"][c]),
        }
        for q in range(N_WIN):
            xq = xc[lay["node_at"][c, q]]
            m[f"xTq{q}"] = np.ascontiguousarray(xq.T)
        in_maps.append(m)

    res = bass_utils.run_bass_kernel_spmd(
        nc, in_maps, core_ids=list(range(N_CORES)), trace=trace
    )
    outs = np.concatenate([r["out"] for r in res.results], axis=0)
    final = outs[lay["old2new"][: n_nodes]]
    return final, res


def kernel(x, edge_index, W_mlp, b_mlp, W1, a1_src, a1_dst, b1,
           W2, a2_src, a2_dst, b2, trace=False, _ret_res=False):
    x = np.asarray(x)
    lay = _build_layout(edge_index, x.shape[0])
    nc = _build_program(lay, x.shape[1], W_mlp.shape[1], W1.shape[1])
    out, res = _run(nc, lay, x, W_mlp, b_mlp, W1, a1_src, a1_dst, b1,
                    W2, a2_src, a2_dst, b2, trace=trace)
    if _ret_res:
        return out, res
    return out


# revision 23
# speedup vs baseline: 1.2622x; 1.0457x over previous
"""GAT-style 2-conv GNN forward on 8 Trainium2 NeuronCores.

Strategy (graph/data parallel): nodes partitioned across 8 cores by
destination range; per-edge source-row gathers via InstDMAGatherAnt (int16
indices) against a replicated node-feature table.

Perf notes vs. the first working version (3.14 ms):
  - dma_gather costs ~7.8 ns per index of serialized GPSIMD time; total
    gathered indices are the kernel's critical resource.
  - d-values (per-dst attention bias) are RECOMPUTED per window order from
    host-permuted x on the idle TensorEngine instead of being gathered
    (-50k indices/core).
  - Windows are local row-quarters (not core pairs), so the table AllGather
    splits into 4 sub-collectives pipelined against window processing.
  - Pad rows are distributed per-quarter so each window table has dummy
    rows for grid padding.
"""

import sys

sys.path.insert(0, "/opt/trn_rl_repo")

import contextlib

import numpy as np

import concourse.bacc as bacc
import concourse.bass as bass
import concourse.bass_utils as bass_utils
import concourse.mybir as mybir
import concourse.tile as tile
from concourse import library_config
from concourse.masks import make_identity

FP32 = mybir.dt.float32
FP16 = mybir.dt.float16
INT16 = mybir.dt.int16

N_CORES = 8
N_WIN = 4
P = 128
ELEM = 128  # fp16 cols per table row = 256B

# table row columns
S1C, S2C, D1C, D2C = 32, 65, 66, 67
H1B, H2B = 0, 33
# partial row columns
PN1, PN2, PD1, PD2 = 0, 32, 64, 65
NEG_SLOPE = 0.2
DUMMY_S = -30000.0

S_CHUNK = 96  # grid columns per chunk (12288 idxs: descriptor-ring limit)
QTILES = (25, 25, 24, 24)  # tiles per window-quarter (sum = 98)


def _wrap16(stream):
    """[n] -> [128, n//16] int16 in the 16-partition wrapped+replicated layout."""
    n = stream.shape[0]
    assert n % 16 == 0
    w = stream.reshape(n // 16, 16).T.astype(np.int16)  # [16, n//16]
    return np.tile(w, (8, 1))


def _build_layout(edge_index, n_nodes):
    src = np.asarray(edge_index[0], dtype=np.int64)
    dst = np.asarray(edge_index[1], dtype=np.int64)
    E = src.shape[0]

    npc_raw = -(-n_nodes // N_CORES)
    tiles = -(-npc_raw // P)
    npc = tiles * P
    if npc == npc_raw:  # ensure pad rows exist
        tiles += 1
        npc += P
    n_pads = npc - npc_raw
    assert tiles == sum(QTILES)

    # quarter row layout: each quarter gets some pad rows at its end
    qrows = np.array(QTILES, dtype=np.int64) * P          # rows per quarter
    qrow_start = np.concatenate([[0], np.cumsum(qrows)])  # [5]
    padq = np.full(N_WIN, n_pads // N_WIN, dtype=np.int64)
    padq[: n_pads % N_WIN] += 1
    qreal = qrows - padq                                   # real rows/quarter
    qreal_start = np.concatenate([[0], np.cumsum(qreal)])  # over raw index
    assert qreal_start[-1] == npc_raw

    wsize = (qrows * N_CORES).astype(np.int64)             # table rows/window
    assert (wsize <= 32768).all()
    pad_rel = qreal.copy()  # core-0 pad start, window-relative

    # old local offset -> new local row
    off = np.arange(npc_raw)
    oq = np.searchsorted(qreal_start[1:], off, side="right")
    new_local = qrow_start[oq] + (off - qreal_start[oq])

    old2new = np.empty(n_nodes, dtype=np.int64)
    for c in range(N_CORES):
        lo = c * npc_raw
        hi = min(lo + npc_raw, n_nodes)
        old2new[lo:hi] = c * npc + new_local[: hi - lo]

    new_src = old2new[src]
    new_dst = old2new[dst]
    dst_core = new_dst // npc
    dst_local = new_dst % npc
    src_core = new_src // npc
    src_local = new_src % npc
    src_win = np.searchsorted(qrow_start[1:4], src_local, side="right")
    # window-table index of each edge's source
    src_tab = src_core * qrows[src_win] + (src_local - qrow_start[src_win])

    # per (core, window) in-degree
    qdeg = np.zeros((N_CORES, N_WIN, npc), dtype=np.int64)
    np.add.at(qdeg, (dst_core, src_win, dst_local), 1)

    node_at = np.empty((N_CORES, N_WIN, npc), dtype=np.int64)
    pos_of = np.empty((N_CORES, N_WIN, npc), dtype=np.int64)
    for c in range(N_CORES):
        for q in range(N_WIN):
            o = np.argsort(-qdeg[c, q], kind="stable")
            node_at[c, q] = o
            pos_of[c, q, o] = np.arange(npc)

    # shared tile degree profile per window
    D_q = np.zeros((N_WIN, tiles), dtype=np.int64)
    for q in range(N_WIN):
        sorted_deg = np.take_along_axis(qdeg[:, q, :], node_at[:, q, :], axis=1)
        D_q[q] = sorted_deg[:, ::P].max(axis=0)

    fb_q = np.zeros((N_WIN, tiles + 1), dtype=np.int64)
    for q in range(N_WIN):
        fb_q[q, 1:] = np.cumsum(D_q[q])
    slots_q = fb_q[:, -1].copy()

    # edge -> grid cell
    pos = pos_of[dst_core, src_win, dst_local]
    t = pos // P
    p = pos % P
    key = (dst_core * N_WIN + src_win) * npc + dst_local
    order = np.argsort(key, kind="stable")
    sk = key[order]
    first = np.flatnonzero(np.r_[True, sk[1:] != sk[:-1]])
    group_start = np.repeat(first, np.diff(np.r_[first, E]))
    j = np.empty(E, dtype=np.int64)
    j[order] = np.arange(E) - group_start
    assert (j < D_q[src_win, t]).all()
    col = fb_q[src_win, t] + j

    gidx = np.empty((N_CORES, P, int(slots_q.sum()) * 8), dtype=np.int16)
    wbase = np.concatenate([[0], np.cumsum(slots_q)])
    for c in range(N_CORES):
        for q in range(N_WIN):
            sq = int(slots_q[q])
            stream = np.full(sq * P, pad_rel[q], dtype=np.int64)
            m = (dst_core == c) & (src_win == q)
            stream[col[m] * P + p[m]] = src_tab[m]
            assert stream.max() < wsize[q] and stream.min() >= 0
            gidx[c, :, int(wbase[q]) * 8 : int(wbase[q] + sq) * 8] = _wrap16(
                stream
            )

    # merge-gather index streams (partial q-order -> common order)
    mgidx = np.empty((N_CORES, P, N_WIN * npc // 16), dtype=np.int16)
    for c in range(N_CORES):
        for q in range(N_WIN):
            sl = slice(q * npc // 16, (q + 1) * npc // 16)
            mgidx[c, :, sl] = _wrap16(pos_of[c, q])

    # chunk structure per window
    win_chunks = []
    for q in range(N_WIN):
        runs = []
        t0 = 0
        for tt in range(1, tiles + 1):
            if tt == tiles or D_q[q, tt] != D_q[q, t0]:
                if D_q[q, t0] > 0:
                    runs.append((t0, tt - t0, int(D_q[q, t0])))
                t0 = tt
        pieces = []
        for (rt0, g, d) in runs:
            max_g = max(1, S_CHUNK // d)
            s = 0
            while s < g:
                gg = min(max_g, g - s)
                pieces.append((rt0 + s, gg, d))
                s += gg
        chunks = []
        cur, cur_cols = [], 0
        for pc in pieces:
            need = pc[1] * pc[2]
            assert need <= S_CHUNK
            if cur_cols + need > S_CHUNK:
                chunks.append(cur)
                cur, cur_cols = [], 0
            cur.append(pc)
            cur_cols += need
        if cur:
            chunks.append(cur)
        win_chunks.append(chunks)

    # pad-row mask in common order [P, tiles]
    padm = np.zeros((npc,), dtype=np.float16)
    for q in range(N_WIN):
        padm[qrow_start[q] + qreal[q] : qrow_start[q + 1]] = DUMMY_S
    padm = np.ascontiguousarray(padm.reshape(tiles, P).T)

    return dict(
        npc_raw=npc_raw, npc=npc, tiles=tiles, wsize=wsize, qrows=qrows,
        qrow_start=qrow_start, old2new=old2new, D_q=D_q, fb_q=fb_q,
        slots_q=slots_q, gidx=gidx, mgidx=mgidx, win_chunks=win_chunks,
        node_at=node_at, padm=padm,
    )


def _build_program(lay, f_in, hidden, ncls):
    tiles = lay["tiles"]
    npc = lay["npc"]
    wsize = lay["wsize"]
    qrow_start = lay["qrow_start"]
    slots_q = lay["slots_q"]
    fb_q = lay["fb_q"]
    win_chunks = lay["win_chunks"]
    F = 2 * ncls + 1
    assert F == f_in
    HC = 2 * ncls + 4

    nc = bacc.Bacc("TRN2", target_bir_lowering=False, debug=False,
                   enable_asserts=False, num_devices=N_CORES,
                   num_swdge_queues=2)

    xT_in = nc.dram_tensor("xT", [f_in, npc], FP32, kind="ExternalInput").ap()
    xTq_in = [
        nc.dram_tensor(f"xTq{q}", [f_in, npc], FP32, kind="ExternalInput").ap()
        for q in range(N_WIN)
    ]
    x_in = nc.dram_tensor("xrow", [npc, f_in], FP32, kind="ExternalInput").ap()
    wmlp_in = nc.dram_tensor("wmlp", [f_in, hidden], FP32, kind="ExternalInput").ap()
    bmlp_in = nc.dram_tensor("bmlp", [hidden, 1], FP32, kind="ExternalInput").ap()
    wcat_in = nc.dram_tensor("wcat", [hidden, HC], FP32, kind="ExternalInput").ap()
    wd_in = nc.dram_tensor("wd", [hidden, 2], FP32, kind="ExternalInput").ap()
    bb_in = nc.dram_tensor("bb", [P, 2 * ncls], FP32, kind="ExternalInput").ap()
    padm_in = nc.dram_tensor("padm", [P, tiles], FP16, kind="ExternalInput").ap()
    gidx_in = nc.dram_tensor(
        "gidx", [P, int(slots_q.sum()) * 8], INT16, kind="ExternalInput"
    ).ap()
    mgidx_in = nc.dram_tensor(
        "mgidx", [P, N_WIN * npc // 16], INT16, kind="ExternalInput"
    ).ap()
    out_t = nc.dram_tensor("out", [npc, F], FP32, kind="ExternalOutput").ap()

    with tile.TileContext(nc) as tc:
        with contextlib.ExitStack() as ctx:
            persist = ctx.enter_context(tc.tile_pool(name="persist", bufs=1))
            dram = ctx.enter_context(tc.tile_pool(name="dram", bufs=1, space="DRAM"))
            cpool = ctx.enter_context(tc.tile_pool(name="consts", bufs=1))

            nc.gpsimd.load_library(library_config.mlp)

            x3buf = persist.tile([P, tiles], FP32)
            bb_sb = persist.tile([P, 2 * ncls], FP32)
            dq_sb = persist.tile([P, N_WIN, tiles, 2], FP32)
            nc.sync.dma_start(out=bb_sb[:], in_=bb_in[:])

            hloc_d = [
                dram.tile([int(lay["qrows"][q]), ELEM], FP16,
                          name=f"hloc{q}", tag=f"hloc{q}")
                for q in range(N_WIN)
            ]
            htab_d = [
                dram.tile([int(wsize[q]), ELEM], FP16, name=f"htab{q}",
                          tag=f"htab{q}")
                for q in range(N_WIN)
            ]
            part_d = [dram.tile([npc, ELEM], FP16, name=f"part{q}",
                                tag=f"part{q}")
                      for q in range(N_WIN)]

            wmlp_sb = cpool.tile([f_in, hidden], FP32)
            bmlp_sb = cpool.tile([hidden, 1], FP32)
            wd_sb = cpool.tile([hidden, 2], FP32)
            ident = cpool.tile([P, P], FP32)
            nc.sync.dma_start(out=wmlp_sb[:], in_=wmlp_in[:])
            nc.sync.dma_start(out=bmlp_sb[:], in_=bmlp_in[:])
            nc.sync.dma_start(out=wd_sb[:], in_=wd_in[:])
            make_identity(nc, ident[:])

            # Phase-3 pools created first so their SBUF ranges don't overlap
            # phase-1/2 tiles (address reuse adds a WAR dep that stalls the
            # first gather until all of phase 1 retires).
            ph3stack = contextlib.ExitStack()
            gip = ph3stack.enter_context(tc.tile_pool(name="gi", bufs=1))
            msgp = ph3stack.enter_context(tc.tile_pool(name="msg", bufs=4))
            scp = ph3stack.enter_context(tc.tile_pool(name="sc", bufs=2))
            exp_ = ph3stack.enter_context(tc.tile_pool(name="exb", bufs=2))
            ptp = ph3stack.enter_context(tc.tile_pool(name="pt", bufs=2))
            gidx_sb = gip.tile([P, int(slots_q.sum()) * 8], INT16)
            nc.sync.dma_start(out=gidx_sb[:], in_=gidx_in[:])

            # ---------------- Phase 1: dense local features ----------------
            with tc.tile_pool(name="ph1c", bufs=1) as c1pool, \
                 tc.tile_pool(name="ph1", bufs=3) as ph1, \
                 tc.tile_pool(name="ph1x", bufs=2) as ph1x, \
                 tc.tile_pool(name="hl", bufs=1) as hlp, \
                 tc.tile_pool(name="ps1", bufs=2, space="PSUM") as ps1, \
                 tc.tile_pool(name="ps2", bufs=2, space="PSUM") as ps2, \
                 tc.tile_pool(name="ps3", bufs=2, space="PSUM") as ps3:
                wcat_sb = c1pool.tile([hidden, HC], FP32)
                padm_sb = c1pool.tile([P, tiles], FP16)
                nc.sync.dma_start(out=wcat_sb[:], in_=wcat_in[:])
                nc.sync.dma_start(out=padm_sb[:], in_=padm_in[:])

                hq_sb = [
                    hlp.tile([P, max(QTILES), ELEM], FP16, name=f"hq{q}",
                             tag=f"hq{q % 2}", bufs=1)[:, : QTILES[q], :]
                    for q in range(N_WIN)
                ]
                for q in range(N_WIN):
                    nc.vector.memset(hq_sb[q][:], 0.0)

                XCH = 4
                q_next = 0
                for t in range(tiles):
                    if t % XCH == 0:
                        g = min(XCH, tiles - t)
                        xt_sb = ph1x.tile([f_in, XCH * P], FP32, tag="xt")
                        nc.sync.dma_start(
                            out=xt_sb[:, : g * P],
                            in_=xT_in[:, t * P : (t + g) * P],
                        )
                    xoff = (t % XCH) * P
                    psA = ps1.tile([P, P], FP32, space="PSUM")
                    nc.tensor.matmul(
                        out=psA[:], lhsT=wmlp_sb[:],
                        rhs=xt_sb[:, xoff : xoff + P],
                        start=True, stop=True,
                    )
                    x0t = ph1.tile([P, P], FP32, tag="x0t")
                    nc.scalar.activation(
                        out=x0t[:], in_=psA[:],
                        func=mybir.ActivationFunctionType.Relu,
                        bias=bmlp_sb[:, 0:1], scale=1.0,
                    )
                    psH = ps2.tile([P, HC], FP32, space="PSUM")
                    nc.tensor.matmul(
                        out=psH[:], lhsT=x0t[:], rhs=wcat_sb[:],
                        start=True, stop=True,
                    )
                    nc.vector.tensor_copy(
                        out=hq_sb[q_next][:, t - int(qrow_start[q_next] // P), 0:HC],
                        in_=psH[:],
                    )
                    psT = ps3.tile([P, P], FP32, space="PSUM")
                    nc.tensor.transpose(out=psT[:], in_=x0t[:], identity=ident[:])
                    nc.vector.tensor_reduce(
                        out=x3buf[:, t : t + 1], in_=psT[:],
                        axis=mybir.AxisListType.X, op=mybir.AluOpType.max,
                    )
                    # quarter complete -> mask pads, write to HBM
                    if t + 1 == qrow_start[q_next + 1] // P:
                        tb = int(qrow_start[q_next] // P)
                        te = t + 1
                        for scol in (S1C, S2C):
                            nc.vector.tensor_tensor(
                                out=hq_sb[q_next][:, :, scol : scol + 1],
                                in0=hq_sb[q_next][:, :, scol : scol + 1],
                                in1=padm_sb[:, tb:te].unsqueeze(2),
                                op=mybir.AluOpType.add,
                            )
                        nc.sync.dma_start(
                            out=hloc_d[q_next][:].rearrange(
                                "(t p) c -> p t c", p=P
                            ),
                            in_=hq_sb[q_next][:],
                        )
                        q_next += 1

            # ------------- Phase 2: recompute d per window order -----------
            with tc.tile_pool(name="ph2x", bufs=2) as ph2x, \
                 tc.tile_pool(name="ph2", bufs=3) as ph2, \
                 tc.tile_pool(name="psA2", bufs=2, space="PSUM") as psa2, \
                 tc.tile_pool(name="psD2", bufs=2, space="PSUM") as psd2:
                XCH = 16
                for q in range(N_WIN):
                    for t in range(tiles):
                        if t % XCH == 0:
                            g = min(XCH, tiles - t)
                            xq_sb = ph2x.tile([f_in, XCH * P], FP32, tag="xq")
                            nc.sync.dma_start(
                                out=xq_sb[:, : g * P],
                                in_=xTq_in[q][:, t * P : (t + g) * P],
                            )
                        xoff = (t % XCH) * P
                        psA = psa2.tile([P, P], FP32, space="PSUM")
                        nc.tensor.matmul(
                            out=psA[:], lhsT=wmlp_sb[:],
                            rhs=xq_sb[:, xoff : xoff + P],
                            start=True, stop=True,
                        )
                        x0q = ph2.tile([P, P], FP32, tag="x0q")
                        nc.scalar.activation(
                            out=x0q[:], in_=psA[:],
                            func=mybir.ActivationFunctionType.Relu,
                            bias=bmlp_sb[:, 0:1], scale=1.0,
                        )
                        psD = psd2.tile([P, 2], FP32, space="PSUM")
                        nc.tensor.matmul(
                            out=psD[:], lhsT=x0q[:], rhs=wd_sb[:],
                            start=True, stop=True,
                        )
                        nc.vector.tensor_copy(
                            out=dq_sb[:, q, t, :], in_=psD[:]
                        )

            # ---------------- Phase 3: per-window gather + reduce ----------
            if True:
                wbase = np.concatenate([[0], np.cumsum(slots_q)]).astype(int)

                def _trigger_ag(q):
                    nc.gpsimd.collective_compute(
                        "AllGather",
                        mybir.AluOpType.bypass,
                        replica_groups=[list(range(N_CORES))],
                        ins=[hloc_d[q][:].opt()],
                        outs=[htab_d[q][:].opt()],
                    )

                _trigger_ag(0)
                for q in range(N_WIN):
                    d1q = dq_sb[:, q, :, 0]
                    d2q = dq_sb[:, q, :, 1]

                    partial = ptp.tile([P, tiles, ELEM], FP16, tag="partial")
                    nc.vector.memset(partial[:], 0.0)

                    tab = htab_d[q][:]
                    for ci, chunk in enumerate(win_chunks[q]):
                        ccols = sum(g * d for (_, g, d) in chunk)
                        cb = int(fb_q[q, chunk[0][0]])
                        msg = msgp.tile([P, S_CHUNK, ELEM], FP16, tag="msg")
                        ib = (int(wbase[q]) + cb) * 8
                        nc.gpsimd.dma_gather(
                            out_ap=msg[:, :ccols, :],
                            in_ap=tab,
                            idxs_ap=gidx_sb[:, ib : ib + ccols * 8],
                            num_idxs=ccols * P,
                            num_idxs_reg=ccols * P,
                            elem_size=ELEM,
                            single_packet=False,
                            queue_num=ci % 2,
                        )
                        if ci == 0 and q + 1 < N_WIN:
                            _trigger_ag(q + 1)
                        for (rt0, g, d) in chunk:
                            s0 = int(fb_q[q, rt0]) - cb
                            mv = msg[:, s0 : s0 + g * d, :].rearrange(
                                "p (g e) c -> p g e c", e=d
                            )
                            for (hb, sc_, dq_, nb, db) in (
                                (H1B, S1C, d1q, PN1, PD1),
                                (H2B, S2C, d2q, PN2, PD2),
                            ):
                                exb = exp_.tile([P, S_CHUNK], FP16, tag="exb")
                                exv = exb[:, : g * d].rearrange(
                                    "p (g e) -> p g e", e=d
                                )
                                nc.vector.tensor_tensor(
                                    out=exv,
                                    in0=mv[:, :, :, sc_],
                                    in1=dq_[:, rt0 : rt0 + g]
                                    .unsqueeze(2)
                                    .broadcast_to([P, g, d]),
                                    op=mybir.AluOpType.add,
                                )
                                lrb = exp_.tile([P, S_CHUNK], FP16, tag="lrb")
                                lrv = lrb[:, : g * d].rearrange(
                                    "p (g e) -> p g e", e=d
                                )
                                nc.vector.tensor_scalar_mul(lrv, exv, NEG_SLOPE)
                                nc.vector.tensor_tensor(
                                    out=exv, in0=exv, in1=lrv,
                                    op=mybir.AluOpType.max,
                                )
                                nc.scalar.activation(
                                    out=exv, in_=exv,
                                    func=mybir.ActivationFunctionType.Exp,
                                )
                                sct = scp.tile(
                                    [P, S_CHUNK, ncls], FP16, tag="sc"
                                )
                                scv = sct[:, : g * d, :].rearrange(
                                    "p (g e) c -> p g e c", e=d
                                )
                                nc.vector.tensor_tensor(
                                    out=scv,
                                    in0=mv[:, :, :, hb : hb + ncls],
                                    in1=exv.unsqueeze(3).broadcast_to(
                                        [P, g, d, ncls]
                                    ),
                                    op=mybir.AluOpType.mult,
                                )
                                with nc.allow_low_precision("fp16 partials"):
                                    nc.vector.tensor_reduce(
                                        out=partial[:, rt0 : rt0 + g, nb : nb + ncls],
                                        in_=sct[:, : g * d, :].rearrange(
                                            "p (g e) c -> p g c e", e=d
                                        ),
                                        axis=mybir.AxisListType.X,
                                        op=mybir.AluOpType.add,
                                    )
                                    nc.vector.tensor_reduce(
                                        out=partial[:, rt0 : rt0 + g, db],
                                        in_=exv,
                                        axis=mybir.AxisListType.X,
                                        op=mybir.AluOpType.add,
                                    )
                    nc.sync.dma_start(
                        out=part_d[q][:].rearrange("(t p) c -> p t c", p=P),
                        in_=partial[:],
                    )

            ph3stack.close()

            # ---------------- Phase 3.9: merge partials --------------------
            with tc.tile_pool(name="mg", bufs=2) as mgp, \
                 tc.tile_pool(name="mgi", bufs=1) as mgip, \
                 tc.tile_pool(name="acc", bufs=1) as accp, \
                 tc.tile_pool(name="fin", bufs=1) as finp, \
                 tc.tile_pool(name="tmp", bufs=1) as tmpp:
                mgidx_sb = mgip.tile([P, N_WIN * npc // 16], INT16)
                nc.sync.dma_start(out=mgidx_sb[:], in_=mgidx_in[:])
                acc = accp.tile([P, tiles, 2 * ncls + 2], FP32)
                for q in range(N_WIN):
                    mg = mgp.tile([P, tiles, ELEM], FP16, tag="mg")
                    ib = q * npc // 16
                    nc.gpsimd.dma_gather(
                        out_ap=mg[:],
                        in_ap=part_d[q][:],
                        idxs_ap=mgidx_sb[:, ib : ib + npc // 16],
                        num_idxs=npc,
                        num_idxs_reg=npc,
                        elem_size=ELEM,
                        single_packet=False,
                        queue_num=q % 2,
                    )
                    if q == 0:
                        nc.vector.tensor_copy(
                            out=acc[:], in_=mg[:, :, 0 : 2 * ncls + 2]
                        )
                    else:
                        nc.vector.tensor_tensor(
                            out=acc[:], in0=acc[:],
                            in1=mg[:, :, 0 : 2 * ncls + 2],
                            op=mybir.AluOpType.add,
                        )

                # ------------- Phase 4: normalize + residual + lsm ---------
                xin = finp.tile([P, tiles, F], FP32)
                nc.sync.dma_start(
                    out=xin[:], in_=x_in[:].rearrange("(t p) f -> p t f", p=P)
                )
                rden = tmpp.tile([P, tiles], FP32, tag="rden")
                for conv in range(2):
                    numv = acc[:, :, conv * ncls : (conv + 1) * ncls]
                    denv = acc[:, :, 2 * ncls + conv]
                    nc.vector.tensor_scalar_add(denv, denv, 1e-16)
                    nc.vector.reciprocal(out=rden[:], in_=denv)
                    nc.vector.tensor_tensor(
                        out=numv, in0=numv,
                        in1=rden[:].unsqueeze(2).broadcast_to([P, tiles, ncls]),
                        op=mybir.AluOpType.mult,
                    )
                    nc.vector.tensor_tensor(
                        out=numv, in0=numv,
                        in1=bb_sb[:, conv * ncls : (conv + 1) * ncls]
                        .unsqueeze(1)
                        .broadcast_to([P, tiles, ncls]),
                        op=mybir.AluOpType.add,
                    )
                    if conv == 0:
                        nc.vector.tensor_scalar_max(numv, numv, 0.0)
                    nc.vector.tensor_tensor(
                        out=xin[:, :, conv * ncls : (conv + 1) * ncls],
                        in0=xin[:, :, conv * ncls : (conv + 1) * ncls],
                        in1=numv,
                        op=mybir.AluOpType.add,
                    )
                nc.vector.tensor_tensor(
                    out=xin[:, :, 2 * ncls], in0=xin[:, :, 2 * ncls],
                    in1=x3buf[:], op=mybir.AluOpType.add,
                )
                mx = tmpp.tile([P, tiles], FP32, tag="mx")
                nc.vector.tensor_reduce(
                    out=mx[:], in_=xin[:], axis=mybir.AxisListType.X,
                    op=mybir.AluOpType.max,
                )
                nc.vector.tensor_tensor(
                    out=xin[:], in0=xin[:],
                    in1=mx[:].unsqueeze(2).broadcast_to([P, tiles, F]),
                    op=mybir.AluOpType.subtract,
                )
                et = tmpp.tile([P, tiles, F], FP32, tag="et")
                nc.scalar.activation(
                    out=et[:], in_=xin[:],
                    func=mybir.ActivationFunctionType.Exp,
                )
                sm = tmpp.tile([P, tiles], FP32, tag="sm")
                nc.vector.tensor_reduce(
                    out=sm[:], in_=et[:], axis=mybir.AxisListType.X,
                    op=mybir.AluOpType.add,
                )
                lg = tmpp.tile([P, tiles], FP32, tag="lg")
                nc.scalar.activation(
                    out=lg[:], in_=sm[:],
                    func=mybir.ActivationFunctionType.Ln,
                )
                nc.vector.tensor_tensor(
                    out=xin[:], in0=xin[:],
                    in1=lg[:].unsqueeze(2).broadcast_to([P, tiles, F]),
                    op=mybir.AluOpType.subtract,
                )
                nc.sync.dma_start(
                    out=out_t[:].rearrange("(t p) f -> p t f", p=P), in_=xin[:]
                )

    nc.compile()
    return nc


def _run(nc, lay, x, W_mlp, b_mlp, W1, a1_src, a1_dst, b1,
         W2, a2_src, a2_dst, b2, trace=False):
    n_nodes, f_in = x.shape
    hidden = W_mlp.shape[1]
    ncls = W1.shape[1]
    npc = lay["npc"]
    n_pad = npc * N_CORES
    HC = 2 * ncls + 4

    xp = np.zeros((n_pad, f_in), dtype=np.float32)
    xp[lay["old2new"][: n_nodes]] = np.asarray(x, dtype=np.float32)

    wcat = np.concatenate(
        [W1, (W1 @ a1_src)[:, None], W2, (W2 @ a2_src)[:, None],
         (W1 @ a1_dst)[:, None], (W2 @ a2_dst)[:, None]],
        axis=1,
    ).astype(np.float32)
    assert wcat.shape == (hidden, HC)
    wd = np.stack([W1 @ a1_dst, W2 @ a2_dst], axis=1).astype(np.float32)
    bb = np.broadcast_to(
        np.concatenate([b1, b2])[None, :], (P, 2 * ncls)
    ).astype(np.float32).copy()

    in_maps = []
    for c in range(N_CORES):
        xc = xp[c * npc : (c + 1) * npc]
        m = {
            "xT": np.ascontiguousarray(xc.T),
            "xrow": np.ascontiguousarray(xc),
            "wmlp": np.asarray(W_mlp, dtype=np.float32),
            "bmlp": np.asarray(b_mlp, dtype=np.float32)[:, None].copy(),
            "wcat": wcat,
            "wd": wd,
            "bb": bb,
            "padm": lay["padm"],
            "gidx": np.ascontiguousarray(lay["gidx"][c]),
            "mgidx": np.ascontiguousarray(lay["mgidx"][c]),
        }
        for q in range(N_WIN):
            xq = xc[lay["node_at"][c, q]]
            m[f"xTq{q}"] = np.ascontiguousarray(xq.T)
        in_maps.append(m)

    res = bass_utils.run_bass_kernel_spmd(
        nc, in_maps, core_ids=list(range(N_CORES)), trace=trace
    )
    outs = np.concatenate([r["out"] for r in res.results], axis=0)
    final = outs[lay["old2new"][: n_nodes]]
    return final, res


def kernel(x, edge_index, W_mlp, b_mlp, W1, a1_src, a1_dst, b1,
           W2, a2_src, a2_dst, b2, trace=False, _ret_res=False):
    x = np.asarray(x)
    lay = _build_layout(edge_index, x.shape[0])
    nc = _build_program(lay, x.shape[1], W_mlp.shape[1], W1.shape[1])
    out, res = _run(nc, lay, x, W_mlp, b_mlp, W1, a1_src, a1_dst, b1,
                    W2, a2_src, a2_dst, b2, trace=trace)
    if _ret_res:
        return out, res
    return out
